# revision 35
# baseline (speedup 1.0000x reference)
"""AttentiveAggregation (segment softmax-pool) Trainium2 kernel, v2.

Math (per graph g): out_g = sum_v alpha_v H_v,  alpha = softmax_g(e),
  e_v = w_score . tanh(W_proj @ H_v + b_proj).

Key transformations:
 * Global shift: softmax is shift invariant per segment, and
   |e| <= ||w_score||_1 (tanh bounded), so a single global constant
   C = ||w_score||_1 replaces the per-segment max. Then
   out_g = (sum_v a_v [H_v|1])[:D] / (...)[D]  with a_v = exp(e_v - C):
   two segment sums, done as one matmul with a ones-column.
 * Quadrant-windowed segment matmul: segments are grouped into blocks
   of 128 (PSUM accumulator partitions). batch is sorted, so each
   128-node tile spans only a few consecutive segments. The host packs
   nodes so tile t's segments fit [32q(t), 32q(t)+64) for a static
   quadrant schedule q(t); the scatter matmul then uses a [128, 32]
   one-hot lhsT accumulating into PSUM partitions [32q, 32q+32) (PE
   tile_position allows only 32-aligned output bases). Tiles whose
   segments cross into the next quadrant emit one extra matmul (rare).
   The 32-wide one-hot build costs ~1/4 of the full 128-wide one and is
   split between DVE and the otherwise-idle Pool engine.
 * SPMD schedules: one program serves all 8 cores, so q(t) must be
   core-invariant. Blocks are ranked by size; slot k takes ranked
   blocks [8k, 8k+8) (one per core) which share one schedule; per-slot
   tile counts t_k are ragged (saves ~4% DMA vs padding to the max).
 * The projection is computed transposed (h on partitions), so b_proj
   folds into the tanh's per-partition bias for free.
 * Sharding: 8 cores x 16 slots each; cores are fully independent
   (no collectives); host un-permutes the per-core [16,128,128] slabs.
"""

import math

import numpy as np

P = 128                    # partitions / tile node count / D / HS
D = 128
G_SEGS = 16384
SEGS_PER_BLK = 128
NBLK_TOT = G_SEGS // SEGS_PER_BLK   # 128 global blocks
N_CORES = 8
NSLOT = NBLK_TOT // N_CORES         # 16 slots per core
GRP = 16                   # tiles per DMA group
CHUNK = 8                  # tiles per proj-PSUM/tanh chunk
QW = 32                    # quadrant width (PSUM write alignment)

CFG = {
    "ht": "f16",
    "rhs": "f16",
    "act": "f16",
    "oh_dve_mod": 1,       # every Nth one-hot build on DVE, rest on Pool
    "e_op": "pe",          # "pe" | "stt" | "ttr": engine for the score reduce
}

# alpha values are prescaled by exp(PRESCALE_LN) inside the exp bias so the
# smallest per-segment weights stay in f16 normal range; numerator and
# denominator scale together so the final division cancels it exactly.
PRESCALE_LN = 14 * math.log(2.0)

_NP_DT = {"f32": np.float32, "f16": np.float16}


def _np_dt(name):
    if name == "bf16":
        import ml_dtypes
        return ml_dtypes.bfloat16
    return _NP_DT[name]


def _my_dt(name, mybir):
    return {
        "f32": mybir.dt.float32,
        "bf16": mybir.dt.bfloat16,
        "f16": mybir.dt.float16,
    }[name]


def _pack_block(seg_local, q_sched):
    """Greedy pack of one block's nodes onto tiles: tile t may hold nodes
    with local seg in [32*q(t), 32*q(t)+64). Returns list of (pos, take)
    per tile, or None if infeasible."""
    n = len(seg_local)
    pos = 0
    tiles = []
    for t in range(len(q_sched)):
        if pos >= n:
            break
        lo = QW * q_sched[t]
        if seg_local[pos] < lo:
            return None
        hi = np.searchsorted(seg_local, min(lo + 2 * QW, SEGS_PER_BLK),
                             side="left")
        take = int(min(128, hi - pos))
        tiles.append((pos, take))
        pos += take
    if pos < n:
        return None
    return tiles


def _plan(batch):
    """Blocks -> slot plans (quadrant schedules + per-block tilings)."""
    s = np.searchsorted(batch, np.arange(NBLK_TOT + 1, dtype=np.int64)
                        * SEGS_PER_BLK)
    lens = s[1:] - s[:-1]
    order = np.argsort(lens, kind="stable")

    slots = []
    for k in range(NSLOT):
        blks = order[k * N_CORES:(k + 1) * N_CORES]
        maxlen = int(max(1, lens[blks].max()))
        T = int(np.ceil(maxlen / 128)) + 8
        seglists = []
        curves = []
        for b in blks:
            segl = (batch[s[b]:s[b + 1]] - b * SEGS_PER_BLK).astype(np.int64)
            seglists.append(segl)
            sl = segl if len(segl) else np.zeros(1, np.int64)
            idx = np.minimum(np.arange(T) * 128, len(sl) - 1)
            curves.append(sl[idx])
        wmin = np.array(curves).min(axis=0)

        packed = None
        # primary: quadrant schedule from the consensus curve; fallback:
        # all-zero schedule with full-width windows (always feasible)
        for relax in (2, 8, 16, None):
            if relax is None:
                q_sched = np.zeros(T, np.int64)
                full = True
            else:
                w = np.maximum(0, np.minimum(SEGS_PER_BLK - QW, wmin - relax))
                q_sched = np.maximum.accumulate(w // QW).astype(np.int64)
                full = False
            tilings = []
            ok = True
            for segl in seglists:
                tiles = (_pack_block(segl, q_sched) if not full else
                         [(i * 128, int(min(128, len(segl) - i * 128)))
                          for i in range(int(np.ceil(len(segl) / 128)))]
                         or [(0, 0)])
                if tiles is None:
                    ok = False
                    break
                tilings.append(tiles)
            if ok:
                t_k = max(1, max(len(tl) for tl in tilings))
                q_sched = q_sched[:t_k]
                # split flag: any block has a node beyond its tile's quadrant
                split = np.zeros(t_k, bool)
                if not full:
                    for segl, tiles in zip(seglists, tilings):
                        for t, (pos, take) in enumerate(tiles):
                            if take and segl[pos + take - 1] >= \
                                    QW * (q_sched[t] + 1):
                                split[t] = True
                packed = dict(blks=blks, q_sched=q_sched, t_k=t_k,
                              tilings=tilings, split=split, full=full)
                break
        slots.append(packed)
    return s, order, slots


def _prep_inputs(H, batch, W_proj, b_proj, w_score, cfg):
    """Host-side repack. Returns (in_maps, plan_meta)."""
    H = np.ascontiguousarray(H, dtype=np.float32)
    batch = np.asarray(batch).astype(np.int64)
    W_proj = np.asarray(W_proj, dtype=np.float32)
    b_proj = np.asarray(b_proj, dtype=np.float32)
    w_score = np.asarray(w_score, dtype=np.float32)

    c_shift = float(np.abs(w_score).sum())

    s, order, slots = _plan(batch)

    dt_rhs = _np_dt(cfg["rhs"])
    dt_ht = _np_dt(cfg["ht"])
    DC = D + 1                       # H | ones

    in_maps = [dict() for _ in range(N_CORES)]

    for k, sl in enumerate(slots):
        t_k, q_sched = sl["t_k"], sl["q_sched"]
        for c in range(N_CORES):
            b = int(sl["blks"][c])
            tiles = sl["tilings"][c]
            segl = (batch[s[b]:s[b + 1]] - b * SEGS_PER_BLK).astype(np.int64)
            idx = np.full((t_k, 128), -1, np.int64)
            for t, (pos, take) in enumerate(tiles):
                if take:
                    idx[t, :take] = np.arange(s[b] + pos, s[b] + pos + take)
            valid = idx >= 0
            idxc = np.maximum(idx, 0)

            rhs = np.zeros((t_k, 128, DC), np.float32)
            Hg = H[idxc]
            Hg[~valid] = 0.0
            rhs[:, :, :D] = Hg
            rhs[:, :, D] = valid
            bl = np.full((t_k, 128), -1000.0, np.float32)
            for t, (pos, take) in enumerate(tiles):
                if take:
                    bl[t, :take] = segl[pos:pos + take] - QW * q_sched[t]
            m = in_maps[c]
            m[f"rhs{k}"] = np.ascontiguousarray(
                rhs.transpose(1, 0, 2).astype(dt_rhs))
            del rhs
            m[f"bl{k}"] = np.ascontiguousarray(bl.T)
            Hg = H[idxc]
            Hg[~valid] = 0.0
            m[f"ht{k}"] = np.ascontiguousarray(
                Hg.transpose(2, 0, 1).astype(dt_ht))
            del Hg

    wt = np.ascontiguousarray(W_proj.T.astype(dt_ht))
    wb = np.ascontiguousarray(
        np.broadcast_to(w_score, (P, D)).astype(_np_dt(cfg["act"])))
    # [h, tt, m]: col tt = w_score, other 31 columns zero - each masked
    # e-matmul then zero-fills its whole 32-aligned PSUM region, so the
    # later exp never reads uninitialized PSUM (a HW fault).
    wmask = np.zeros((P, CHUNK, QW), np.float32)
    for tt in range(CHUNK):
        wmask[:, tt, tt] = w_score
    wmask = np.ascontiguousarray(
        wmask.reshape(P, CHUNK * QW).astype(_np_dt(cfg["act"])))
    eye16 = np.zeros((2 * QW, CHUNK), np.float32)
    eye16[0:CHUNK] = np.eye(CHUNK)
    eye16[QW:QW + CHUNK] = np.eye(CHUNK)
    eye16 = np.ascontiguousarray(eye16.astype(_np_dt(cfg["act"])))
    bb = np.ascontiguousarray(b_proj.reshape(P, 1).astype(np.float32))
    iota_lo = np.ascontiguousarray(
        np.broadcast_to(np.arange(QW, dtype=np.float32), (P, QW))
        .astype(dt_rhs))
    iota_hi = np.ascontiguousarray((iota_lo.astype(np.float32) + QW)
                                   .astype(dt_rhs))
    iota_fl = np.ascontiguousarray(
        np.broadcast_to(np.arange(P, dtype=np.float32), (P, P))
        .astype(dt_rhs))
    for c in range(N_CORES):
        in_maps[c]["wt"] = wt
        in_maps[c]["wb"] = wb
        in_maps[c]["wmask"] = wmask
        in_maps[c]["eye16"] = eye16
        in_maps[c]["iota_lo"] = iota_lo
        in_maps[c]["iota_hi"] = iota_hi
        in_maps[c]["iota_fl"] = iota_fl
        in_maps[c]["bb"] = bb

    meta = dict(slots=[dict(t_k=sl["t_k"], q_sched=sl["q_sched"].tolist(),
                            split=sl["split"].tolist(), full=sl["full"])
                       for sl in slots],
                order=order, c_shift=c_shift)
    return in_maps, meta


def _build_program(meta, cfg):
    import concourse.bacc as bacc
    import concourse.mybir as mybir
    import concourse.tile as tile

    f32 = mybir.dt.float32
    dt_ht = _my_dt(cfg["ht"], mybir)
    dt_rhs = _my_dt(cfg["rhs"], mybir)
    dt_act = _my_dt(cfg["act"], mybir)
    c_shift = meta["c_shift"]
    slots = meta["slots"]
    DC = D + 1
    oh_mod = int(cfg["oh_dve_mod"])

    nc = bacc.Bacc(None, target_bir_lowering=False)

    rhs_d, ht_d = [], []
    for k, sl in enumerate(slots):
        rhs_d.append(nc.dram_tensor(f"rhs{k}", [P, sl["t_k"], DC], dt_rhs,
                                    kind="ExternalInput"))
        ht_d.append(nc.dram_tensor(f"ht{k}", [P, sl["t_k"], D], dt_ht,
                                   kind="ExternalInput"))
    bl_d = [nc.dram_tensor(f"bl{k}", [P, sl["t_k"]], f32,
                           kind="ExternalInput")
            for k, sl in enumerate(slots)]
    wt_d = nc.dram_tensor("wt", [P, D], dt_ht, kind="ExternalInput")
    wb_d = nc.dram_tensor("wb", [P, D], dt_act, kind="ExternalInput")
    wm_d = nc.dram_tensor("wmask", [P, CHUNK * QW], dt_act,
                          kind="ExternalInput")
    ey_d = nc.dram_tensor("eye16", [2 * QW, CHUNK], dt_act,
                          kind="ExternalInput")
    ilo_d = nc.dram_tensor("iota_lo", [P, QW], dt_rhs, kind="ExternalInput")
    ihi_d = nc.dram_tensor("iota_hi", [P, QW], dt_rhs, kind="ExternalInput")
    ifl_d = nc.dram_tensor("iota_fl", [P, P], dt_rhs, kind="ExternalInput")
    bb_d = nc.dram_tensor("bb", [P, 1], f32, kind="ExternalInput")
    out_d = nc.dram_tensor("out", [NSLOT, P, D], f32, kind="ExternalOutput")

    with tile.TileContext(nc) as tc:
        with (
            tc.tile_pool(name="const", bufs=1) as constp,
            tc.tile_pool(name="rhsp", bufs=9) as rhsp,
            tc.tile_pool(name="htp", bufs=8) as htp,
            tc.tile_pool(name="midp", bufs=6) as midp,
            tc.tile_pool(name="ohp", bufs=44) as ohp,
            tc.tile_pool(name="blp", bufs=2) as blp,
            tc.tile_pool(name="pp", bufs=2, space="PSUM") as pp,
            tc.tile_pool(name="op", bufs=1, space="PSUM") as op,
            tc.tile_pool(name="ep", bufs=2, space="PSUM") as ep,
            tc.tile_pool(name="tp", bufs=1, space="PSUM") as tp,
        ):
            wt_sb = constp.tile([P, D], dt_ht)
            nc.sync.dma_start(wt_sb[:], wt_d[:])
            wb_sb = constp.tile([P, D], dt_act)
            nc.sync.dma_start(wb_sb[:], wb_d[:])
            wm_sb = constp.tile([P, CHUNK, QW], dt_act)
            nc.sync.dma_start(wm_sb[:], wm_d[:].rearrange(
                "p (t m) -> p t m", t=CHUNK))
            ey_sb = constp.tile([2 * QW, CHUNK], dt_act)
            nc.sync.dma_start(ey_sb[:], ey_d[:])
            ilo_sb = constp.tile([P, QW], dt_rhs)
            nc.sync.dma_start(ilo_sb[:], ilo_d[:])
            ihi_sb = constp.tile([P, QW], dt_rhs)
            nc.sync.dma_start(ihi_sb[:], ihi_d[:])
            ifl_sb = constp.tile([P, P], dt_rhs)
            nc.sync.dma_start(ifl_sb[:], ifl_d[:])
            zrhs_sb = constp.tile([P, D + 1], dt_rhs)
            nc.gpsimd.memset(zrhs_sb[:], 0.0)
            negc_sb = constp.tile([P, 1], f32)
            nc.gpsimd.memset(negc_sb[:], -float(c_shift) + PRESCALE_LN)
            bb_sb = constp.tile([P, 1], f32)
            nc.sync.dma_start(bb_sb[:], bb_d[:])

            tmax = max(sl["t_k"] for sl in slots)
            tasks = []
            # largest slots first: the pipeline drain then runs on the
            # smallest slot, shortening the tail
            for k in sorted(range(NSLOT), key=lambda k: -slots[k]["t_k"]):
                for g0 in range(0, slots[k]["t_k"], GRP):
                    tasks.append((k, g0, min(GRP, slots[k]["t_k"] - g0)))

            out_ps_by_slot = {}
            bl_by_slot = {}
            oh_ctr = 0

            def emit_exp(pend):
                k, g0, gsz, rhs_sb, e_list = pend
                aTs = []
                for ci, c0, csz, e_ps_c in e_list:
                    aT = midp.tile([QW, P], dt_act, tag="aT")
                    nc.scalar.activation(
                        aT[:QW], e_ps_c[:QW],
                        mybir.ActivationFunctionType.Exp,
                        bias=negc_sb[:QW])
                    aTs.append((ci, c0, csz, aT))
                return (k, g0, gsz, rhs_sb, aTs)

            def emit_weights(pend):
                """transpose+copy of a, then the one-hot builds (1 group
                ahead of the scatter matmuls). Returns oh tile list."""
                nonlocal oh_ctr
                k, g0, gsz, rhs_sb, aTs = pend
                sl = slots[k]
                a_ps = tp.tile([P, GRP], dt_act)
                for ci, c0, csz, aT in aTs:
                    nc.tensor.transpose(a_ps[:, c0:c0 + csz], aT[:csz],
                                        ey_sb[:csz, :csz])
                a16 = midp.tile([P, GRP], f32, tag="a16")
                nc.vector.tensor_copy(a16[:, :gsz], a_ps[:, :gsz])
                bl_sb = bl_by_slot[k]
                ohs = []
                for tt in range(gsz):
                    t = g0 + tt
                    bl_ap = bl_sb[:, t:t + 1]
                    a_ap = a16[:, tt:tt + 1]
                    wide = QW * 4 if sl["full"] else QW
                    iota = ifl_sb if sl["full"] else ilo_sb
                    oh = ohp.tile([P, wide], dt_rhs,
                                  tag="ohf" if sl["full"] else "oh")
                    eng = nc.vector if oh_ctr % oh_mod == 0 else nc.gpsimd
                    oh_ctr += 1
                    eng.tensor_scalar(
                        oh[:], iota[:], bl_ap, a_ap,
                        mybir.AluOpType.is_equal, mybir.AluOpType.mult)
                    oh2 = None
                    q = int(sl["q_sched"][t]) if not sl["full"] else 0
                    if not sl["full"] and bool(sl["split"][t]) and q < 3:
                        oh2 = ohp.tile([P, QW], dt_rhs, tag="oh2")
                        nc.vector.tensor_scalar(
                            oh2[:], ihi_sb[:], bl_ap, a_ap,
                            mybir.AluOpType.is_equal, mybir.AluOpType.mult)
                    ohs.append((oh, oh2))
                return (k, g0, gsz, rhs_sb, ohs)

            def emit_scatter(pend):
                k, g0, gsz, rhs_sb, ohs = pend
                sl = slots[k]
                t_k, q_sched, full = sl["t_k"], sl["q_sched"], sl["full"]
                if g0 == 0:
                    out_ps = op.tile([P, D + 1], f32)
                    out_ps_by_slot[k] = (out_ps, set())
                out_ps, qdone = out_ps_by_slot[k]

                def mm(qbase, qwid, oh, stop):
                    first = qbase not in qdone
                    qdone.add(qbase)
                    nc.tensor.matmul(
                        out_ps[qbase:qbase + qwid, :], oh[:],
                        rhs_sb[:, tt, 0:D + 1],
                        start=first, stop=stop,
                        skip_group_check=True, tile_position=(0, qbase))

                for tt in range(gsz):
                    t = g0 + tt
                    is_last = (t == t_k - 1)
                    oh, oh2 = ohs[tt]
                    if full:
                        mm(0, P, oh, is_last)
                        continue
                    q = int(q_sched[t])
                    mm(QW * q, QW, oh, is_last and oh2 is None)
                    if oh2 is not None:
                        mm(QW * (q + 1), QW, oh2, is_last)
                if g0 + gsz == t_k:
                    if not full:
                        for q in range(4):
                            if QW * q not in qdone:
                                nc.tensor.matmul(
                                    out_ps[QW * q:QW * (q + 1), :],
                                    wt_sb[:, 0:QW], zrhs_sb[:],
                                    start=True, stop=True,
                                    skip_group_check=True,
                                    tile_position=(0, QW * q))
                    elif not qdone:
                        nc.tensor.matmul(out_ps[:], wt_sb[:], zrhs_sb[:],
                                         start=True, stop=True,
                                         skip_group_check=True)
                    den_sb = midp.tile([P, 1], f32, tag="den")
                    nc.vector.tensor_scalar_max(den_sb[:],
                                                out_ps[:, D:D + 1], 1e-12)
                    rec_sb = midp.tile([P, 1], f32, tag="rec")
                    nc.vector.reciprocal(rec_sb[:], den_sb[:])
                    res_sb = midp.tile([P, D], f32, tag="res")
                    nc.vector.tensor_scalar(
                        res_sb[:], out_ps[:, 0:D], rec_sb[:], None,
                        mybir.AluOpType.mult)
                    nc.scalar.dma_start(out_d[k], res_sb[:])

            pend_e = None      # group awaiting weights (exp already done)
            pend_w = None
            for k, g0, gsz in tasks:
                sl = slots[k]
                if g0 == 0:
                    bl_sb = blp.tile([P, tmax], f32, tag="bl")
                    nc.scalar.dma_start(bl_sb[:, :sl["t_k"]], bl_d[k][:])
                    bl_by_slot[k] = bl_sb
                rhs_sb = rhsp.tile([P, GRP, DC], dt_rhs, tag="rhs")
                nc.sync.dma_start(rhs_sb[:, :gsz],
                                  rhs_d[k][:, g0:g0 + gsz, :])
                ht_sb = htp.tile([P, GRP, D], dt_ht, tag="ht")
                nc.sync.dma_start(ht_sb[:, :gsz],
                                  ht_d[k][:, g0:g0 + gsz, :])
                tanh_cs = []
                for ci, c0 in enumerate(range(0, gsz, CHUNK)):
                    csz = min(CHUNK, gsz - c0)
                    proj_ps = pp.tile([P, CHUNK, D], f32)
                    for tt in range(csz):
                        nc.tensor.matmul(proj_ps[:, tt], wt_sb[:],
                                         ht_sb[:, c0 + tt],
                                         start=True, stop=True)
                    tanh_c = midp.tile([P, CHUNK, D], dt_act, tag="tanh")
                    nc.scalar.activation(
                        tanh_c[:, :csz], proj_ps[:, :csz],
                        mybir.ActivationFunctionType.Tanh, bias=bb_sb[:])
                    tanh_cs.append((c0, csz, tanh_c))
                if pend_w is not None:
                    emit_scatter(pend_w)
                    pend_w = None
                e_list = []
                for ci, (c0, csz, tanh_c) in enumerate(tanh_cs):
                    e_ps_c = ep.tile([QW, P], f32, tag="eps")
                    for tt in range(csz):
                        nc.tensor.matmul(
                            e_ps_c[:, :], wm_sb[:, tt], tanh_c[:, tt],
                            start=(tt == 0), stop=(tt == csz - 1),
                            skip_group_check=True)
                    e_list.append((ci, c0, csz, e_ps_c))
                new_e = emit_exp((k, g0, gsz, rhs_sb, e_list))
                # weights of the previous group are built at the tail of
                # this iteration (their exp is long done); their scatter
                # runs next iteration, filling the PE hole under the tanhs
                pend_w = emit_weights(pend_e) if pend_e is not None else None
                pend_e = new_e
            # drain the pipeline
            if pend_w is not None:
                emit_scatter(pend_w)
            if pend_e is not None:
                emit_scatter(emit_weights(pend_e))
    nc.compile()
    return nc


def kernel(H, batch, W_proj, b_proj, w_score):
    from concourse.bass_utils import run_bass_kernel_spmd

    in_maps, meta = _prep_inputs(H, batch, W_proj, b_proj, w_score, CFG)
    nc = _build_program(meta, CFG)
    res = run_bass_kernel_spmd(nc, in_maps, core_ids=list(range(N_CORES)))
    out = np.empty((G_SEGS, D), np.float32)
    order = meta["order"]
    for c in range(N_CORES):
        slab = res.results[c]["out"]          # [NSLOT, P, D]
        for k in range(NSLOT):
            b = int(order[k * N_CORES + c])
            out[b * P:(b + 1) * P] = slab[k]
    return out.astype(np.float32)


# revision 41
# speedup vs baseline: 1.0323x; 1.0323x over previous
"""AttentiveAggregation (segment softmax-pool) Trainium2 kernel, v2.

Math (per graph g): out_g = sum_v alpha_v H_v,  alpha = softmax_g(e),
  e_v = w_score . tanh(W_proj @ H_v + b_proj).

Key transformations:
 * Global shift: softmax is shift invariant per segment, and
   |e| <= ||w_score||_1 (tanh bounded), so a single global constant
   C = ||w_score||_1 replaces the per-segment max. Then
   out_g = (sum_v a_v [H_v|1])[:D] / (...)[D]  with a_v = exp(e_v - C):
   two segment sums, done as one matmul with a ones-column.
 * Quadrant-windowed segment matmul: segments are grouped into blocks
   of 128 (PSUM accumulator partitions). batch is sorted, so each
   128-node tile spans only a few consecutive segments. The host packs
   nodes so tile t's segments fit [32q(t), 32q(t)+64) for a static
   quadrant schedule q(t); the scatter matmul then uses a [128, 32]
   one-hot lhsT accumulating into PSUM partitions [32q, 32q+32) (PE
   tile_position allows only 32-aligned output bases). Tiles whose
   segments cross into the next quadrant emit one extra matmul (rare).
   The 32-wide one-hot build costs ~1/4 of the full 128-wide one and is
   split between DVE and the otherwise-idle Pool engine.
 * SPMD schedules: one program serves all 8 cores, so q(t) must be
   core-invariant. Blocks are ranked by size; slot k takes ranked
   blocks [8k, 8k+8) (one per core) which share one schedule; per-slot
   tile counts t_k are ragged (saves ~4% DMA vs padding to the max).
 * The projection is computed transposed (h on partitions), so b_proj
   folds into the tanh's per-partition bias for free.
 * Sharding: 8 cores x 16 slots each; cores are fully independent
   (no collectives); host un-permutes the per-core [16,128,128] slabs.
"""

import math

import numpy as np

P = 128                    # partitions / tile node count / D / HS
D = 128
G_SEGS = 16384
SEGS_PER_BLK = 128
NBLK_TOT = G_SEGS // SEGS_PER_BLK   # 128 global blocks
N_CORES = 8
NSLOT = NBLK_TOT // N_CORES         # 16 slots per core
GRP = 24                   # tiles per DMA group
CHUNK = 8                  # tiles per proj-PSUM/tanh chunk
QW = 32                    # quadrant width (PSUM write alignment)

CFG = {
    "ht": "f16",
    "rhs": "f16",
    "act": "f16",
    "oh_dve_mod": 1,       # every Nth one-hot build on DVE, rest on Pool
    "e_op": "pe",          # "pe" | "stt" | "ttr": engine for the score reduce
}

# alpha values are prescaled by exp(PRESCALE_LN) inside the exp bias so the
# smallest per-segment weights stay in f16 normal range; numerator and
# denominator scale together so the final division cancels it exactly.
PRESCALE_LN = 14 * math.log(2.0)

_NP_DT = {"f32": np.float32, "f16": np.float16}


def _np_dt(name):
    if name == "bf16":
        import ml_dtypes
        return ml_dtypes.bfloat16
    return _NP_DT[name]


def _my_dt(name, mybir):
    return {
        "f32": mybir.dt.float32,
        "bf16": mybir.dt.bfloat16,
        "f16": mybir.dt.float16,
    }[name]


def _pack_block(seg_local, q_sched):
    """Greedy pack of one block's nodes onto tiles: tile t may hold nodes
    with local seg in [32*q(t), 32*q(t)+64). Returns list of (pos, take)
    per tile, or None if infeasible."""
    n = len(seg_local)
    pos = 0
    tiles = []
    for t in range(len(q_sched)):
        if pos >= n:
            break
        lo = QW * q_sched[t]
        if seg_local[pos] < lo:
            return None
        hi = np.searchsorted(seg_local, min(lo + 2 * QW, SEGS_PER_BLK),
                             side="left")
        take = int(min(128, hi - pos))
        tiles.append((pos, take))
        pos += take
    if pos < n:
        return None
    return tiles


def _plan(batch):
    """Blocks -> slot plans (quadrant schedules + per-block tilings)."""
    s = np.searchsorted(batch, np.arange(NBLK_TOT + 1, dtype=np.int64)
                        * SEGS_PER_BLK)
    lens = s[1:] - s[:-1]
    order = np.argsort(lens, kind="stable")

    slots = []
    for k in range(NSLOT):
        blks = order[k * N_CORES:(k + 1) * N_CORES]
        maxlen = int(max(1, lens[blks].max()))
        T = int(np.ceil(maxlen / 128)) + 8
        seglists = []
        curves = []
        for b in blks:
            segl = (batch[s[b]:s[b + 1]] - b * SEGS_PER_BLK).astype(np.int64)
            seglists.append(segl)
            sl = segl if len(segl) else np.zeros(1, np.int64)
            idx = np.minimum(np.arange(T) * 128, len(sl) - 1)
            curves.append(sl[idx])
        wmin = np.array(curves).min(axis=0)

        packed = None
        # primary: quadrant schedule from the consensus curve; fallback:
        # all-zero schedule with full-width windows (always feasible)
        for relax in (2, 8, 16, None):
            if relax is None:
                q_sched = np.zeros(T, np.int64)
                full = True
            else:
                w = np.maximum(0, np.minimum(SEGS_PER_BLK - QW, wmin - relax))
                q_sched = np.maximum.accumulate(w // QW).astype(np.int64)
                full = False
            tilings = []
            ok = True
            for segl in seglists:
                tiles = (_pack_block(segl, q_sched) if not full else
                         [(i * 128, int(min(128, len(segl) - i * 128)))
                          for i in range(int(np.ceil(len(segl) / 128)))]
                         or [(0, 0)])
                if tiles is None:
                    ok = False
                    break
                tilings.append(tiles)
            if ok:
                t_k = max(1, max(len(tl) for tl in tilings))
                q_sched = q_sched[:t_k]
                # split flag: any block has a node beyond its tile's quadrant
                split = np.zeros(t_k, bool)
                if not full:
                    for segl, tiles in zip(seglists, tilings):
                        for t, (pos, take) in enumerate(tiles):
                            if take and segl[pos + take - 1] >= \
                                    QW * (q_sched[t] + 1):
                                split[t] = True
                packed = dict(blks=blks, q_sched=q_sched, t_k=t_k,
                              tilings=tilings, split=split, full=full)
                break
        slots.append(packed)
    return s, order, slots


def _prep_inputs(H, batch, W_proj, b_proj, w_score, cfg):
    """Host-side repack. Returns (in_maps, plan_meta)."""
    H = np.ascontiguousarray(H, dtype=np.float32)
    batch = np.asarray(batch).astype(np.int64)
    W_proj = np.asarray(W_proj, dtype=np.float32)
    b_proj = np.asarray(b_proj, dtype=np.float32)
    w_score = np.asarray(w_score, dtype=np.float32)

    c_shift = float(np.abs(w_score).sum())

    s, order, slots = _plan(batch)

    dt_rhs = _np_dt(cfg["rhs"])
    dt_ht = _np_dt(cfg["ht"])
    DC = D + 1                       # H | ones
    H16 = H.astype(dt_rhs)

    in_maps = [dict() for _ in range(N_CORES)]

    for k, sl in enumerate(slots):
        t_k, q_sched = sl["t_k"], sl["q_sched"]
        for c in range(N_CORES):
            b = int(sl["blks"][c])
            tiles = sl["tilings"][c]
            segl = (batch[s[b]:s[b + 1]] - b * SEGS_PER_BLK).astype(np.int64)
            idx = np.full((t_k, 128), -1, np.int64)
            for t, (pos, take) in enumerate(tiles):
                if take:
                    idx[t, :take] = np.arange(s[b] + pos, s[b] + pos + take)
            valid = idx >= 0
            idxc = np.maximum(idx, 0)

            Hg = H16[idxc]
            Hg[~valid] = 0.0
            rhs = np.zeros((t_k, 128, DC), dt_rhs)
            rhs[:, :, :D] = Hg
            rhs[:, :, D] = valid
            bl = np.full((t_k, 128), -1000.0, np.float32)
            for t, (pos, take) in enumerate(tiles):
                if take:
                    bl[t, :take] = segl[pos:pos + take] - QW * q_sched[t]
            m = in_maps[c]
            m[f"rhs{k}"] = np.ascontiguousarray(rhs.transpose(1, 0, 2))
            del rhs
            m[f"bl{k}"] = np.ascontiguousarray(bl.T)
            m[f"ht{k}"] = np.ascontiguousarray(Hg.transpose(2, 0, 1))
            del Hg

    wt = np.ascontiguousarray(W_proj.T.astype(dt_ht))
    wb = np.ascontiguousarray(
        np.broadcast_to(w_score, (P, D)).astype(_np_dt(cfg["act"])))
    # [h, tt, m]: col tt = w_score, other 31 columns zero - each masked
    # e-matmul then zero-fills its whole 32-aligned PSUM region, so the
    # later exp never reads uninitialized PSUM (a HW fault).
    wmask = np.zeros((P, CHUNK, QW), np.float32)
    for tt in range(CHUNK):
        wmask[:, tt, tt] = w_score
    wmask = np.ascontiguousarray(
        wmask.reshape(P, CHUNK * QW).astype(_np_dt(cfg["act"])))
    eye16 = np.zeros((2 * QW, CHUNK), np.float32)
    eye16[0:CHUNK] = np.eye(CHUNK)
    eye16[QW:QW + CHUNK] = np.eye(CHUNK)
    eye16 = np.ascontiguousarray(eye16.astype(_np_dt(cfg["act"])))
    bb = np.ascontiguousarray(b_proj.reshape(P, 1).astype(np.float32))
    iota_lo = np.ascontiguousarray(
        np.broadcast_to(np.arange(QW, dtype=np.float32), (P, QW))
        .astype(dt_rhs))
    iota_hi = np.ascontiguousarray((iota_lo.astype(np.float32) + QW)
                                   .astype(dt_rhs))
    iota_fl = np.ascontiguousarray(
        np.broadcast_to(np.arange(P, dtype=np.float32), (P, P))
        .astype(dt_rhs))
    for c in range(N_CORES):
        in_maps[c]["wt"] = wt
        in_maps[c]["wb"] = wb
        in_maps[c]["wmask"] = wmask
        in_maps[c]["eye16"] = eye16
        in_maps[c]["iota_lo"] = iota_lo
        in_maps[c]["iota_hi"] = iota_hi
        in_maps[c]["iota_fl"] = iota_fl
        in_maps[c]["bb"] = bb

    meta = dict(slots=[dict(t_k=sl["t_k"], q_sched=sl["q_sched"].tolist(),
                            split=sl["split"].tolist(), full=sl["full"])
                       for sl in slots],
                order=order, c_shift=c_shift)
    return in_maps, meta


def _build_program(meta, cfg):
    import concourse.bacc as bacc
    import concourse.mybir as mybir
    import concourse.tile as tile

    f32 = mybir.dt.float32
    dt_ht = _my_dt(cfg["ht"], mybir)
    dt_rhs = _my_dt(cfg["rhs"], mybir)
    dt_act = _my_dt(cfg["act"], mybir)
    c_shift = meta["c_shift"]
    slots = meta["slots"]
    DC = D + 1
    oh_mod = int(cfg["oh_dve_mod"])

    nc = bacc.Bacc(None, target_bir_lowering=False)

    rhs_d, ht_d = [], []
    for k, sl in enumerate(slots):
        rhs_d.append(nc.dram_tensor(f"rhs{k}", [P, sl["t_k"], DC], dt_rhs,
                                    kind="ExternalInput"))
        ht_d.append(nc.dram_tensor(f"ht{k}", [P, sl["t_k"], D], dt_ht,
                                   kind="ExternalInput"))
    bl_d = [nc.dram_tensor(f"bl{k}", [P, sl["t_k"]], f32,
                           kind="ExternalInput")
            for k, sl in enumerate(slots)]
    wt_d = nc.dram_tensor("wt", [P, D], dt_ht, kind="ExternalInput")
    wb_d = nc.dram_tensor("wb", [P, D], dt_act, kind="ExternalInput")
    wm_d = nc.dram_tensor("wmask", [P, CHUNK * QW], dt_act,
                          kind="ExternalInput")
    ey_d = nc.dram_tensor("eye16", [2 * QW, CHUNK], dt_act,
                          kind="ExternalInput")
    ilo_d = nc.dram_tensor("iota_lo", [P, QW], dt_rhs, kind="ExternalInput")
    ihi_d = nc.dram_tensor("iota_hi", [P, QW], dt_rhs, kind="ExternalInput")
    ifl_d = nc.dram_tensor("iota_fl", [P, P], dt_rhs, kind="ExternalInput")
    bb_d = nc.dram_tensor("bb", [P, 1], f32, kind="ExternalInput")
    out_d = nc.dram_tensor("out", [NSLOT, P, D], f32, kind="ExternalOutput")

    with tile.TileContext(nc) as tc:
        with (
            tc.tile_pool(name="const", bufs=1) as constp,
            tc.tile_pool(name="rhsp", bufs=9) as rhsp,
            tc.tile_pool(name="htp", bufs=8) as htp,
            tc.tile_pool(name="midp", bufs=8) as midp,
            tc.tile_pool(name="ohp", bufs=60) as ohp,
            tc.tile_pool(name="blp", bufs=2) as blp,
            tc.tile_pool(name="pp", bufs=2, space="PSUM") as pp,
            tc.tile_pool(name="op", bufs=1, space="PSUM") as op,
            tc.tile_pool(name="ep", bufs=2, space="PSUM") as ep,
            tc.tile_pool(name="tp", bufs=1, space="PSUM") as tp,
        ):
            wt_sb = constp.tile([P, D], dt_ht)
            nc.sync.dma_start(wt_sb[:], wt_d[:])
            wb_sb = constp.tile([P, D], dt_act)
            nc.sync.dma_start(wb_sb[:], wb_d[:])
            wm_sb = constp.tile([P, CHUNK, QW], dt_act)
            nc.sync.dma_start(wm_sb[:], wm_d[:].rearrange(
                "p (t m) -> p t m", t=CHUNK))
            ey_sb = constp.tile([2 * QW, CHUNK], dt_act)
            nc.sync.dma_start(ey_sb[:], ey_d[:])
            ilo_sb = constp.tile([P, QW], dt_rhs)
            nc.sync.dma_start(ilo_sb[:], ilo_d[:])
            ihi_sb = constp.tile([P, QW], dt_rhs)
            nc.sync.dma_start(ihi_sb[:], ihi_d[:])
            ifl_sb = constp.tile([P, P], dt_rhs)
            nc.sync.dma_start(ifl_sb[:], ifl_d[:])
            zrhs_sb = constp.tile([P, D + 1], dt_rhs)
            nc.gpsimd.memset(zrhs_sb[:], 0.0)
            negc_sb = constp.tile([P, 1], f32)
            nc.gpsimd.memset(negc_sb[:], -float(c_shift) + PRESCALE_LN)
            bb_sb = constp.tile([P, 1], f32)
            nc.sync.dma_start(bb_sb[:], bb_d[:])

            tmax = max(sl["t_k"] for sl in slots)
            tasks = []
            # largest slots first: the pipeline drain then runs on the
            # smallest slot, shortening the tail
            for k in sorted(range(NSLOT), key=lambda k: -slots[k]["t_k"]):
                for g0 in range(0, slots[k]["t_k"], GRP):
                    tasks.append((k, g0, min(GRP, slots[k]["t_k"] - g0)))

            out_ps_by_slot = {}
            bl_by_slot = {}
            oh_ctr = 0

            def emit_exp(pend):
                k, g0, gsz, rhs_sb, e_list = pend
                aTs = []
                for ci, c0, csz, e_ps_c in e_list:
                    aT = midp.tile([QW, P], dt_act, tag="aT")
                    nc.scalar.activation(
                        aT[:QW], e_ps_c[:QW],
                        mybir.ActivationFunctionType.Exp,
                        bias=negc_sb[:QW])
                    aTs.append((ci, c0, csz, aT))
                return (k, g0, gsz, rhs_sb, aTs)

            def emit_weights(pend):
                """transpose+copy of a, then the one-hot builds (1 group
                ahead of the scatter matmuls). Returns oh tile list."""
                nonlocal oh_ctr
                k, g0, gsz, rhs_sb, aTs = pend
                sl = slots[k]
                a_ps = tp.tile([P, GRP], dt_act)
                for ci, c0, csz, aT in aTs:
                    nc.tensor.transpose(a_ps[:, c0:c0 + csz], aT[:csz],
                                        ey_sb[:csz, :csz])
                a16 = midp.tile([P, GRP], f32, tag="a16")
                nc.vector.tensor_copy(a16[:, :gsz], a_ps[:, :gsz])
                bl_sb = bl_by_slot[k]
                ohs = []
                for tt in range(gsz):
                    t = g0 + tt
                    bl_ap = bl_sb[:, t:t + 1]
                    a_ap = a16[:, tt:tt + 1]
                    wide = QW * 4 if sl["full"] else QW
                    iota = ifl_sb if sl["full"] else ilo_sb
                    oh = ohp.tile([P, wide], dt_rhs,
                                  tag="ohf" if sl["full"] else "oh")
                    eng = nc.vector if oh_ctr % oh_mod == 0 else nc.gpsimd
                    oh_ctr += 1
                    eng.tensor_scalar(
                        oh[:], iota[:], bl_ap, a_ap,
                        mybir.AluOpType.is_equal, mybir.AluOpType.mult)
                    oh2 = None
                    q = int(sl["q_sched"][t]) if not sl["full"] else 0
                    if not sl["full"] and bool(sl["split"][t]) and q < 3:
                        oh2 = ohp.tile([P, QW], dt_rhs, tag="oh2")
                        nc.vector.tensor_scalar(
                            oh2[:], ihi_sb[:], bl_ap, a_ap,
                            mybir.AluOpType.is_equal, mybir.AluOpType.mult)
                    ohs.append((oh, oh2))
                return (k, g0, gsz, rhs_sb, ohs)

            def emit_scatter(pend):
                k, g0, gsz, rhs_sb, ohs = pend
                sl = slots[k]
                t_k, q_sched, full = sl["t_k"], sl["q_sched"], sl["full"]
                if g0 == 0:
                    out_ps = op.tile([P, D + 1], f32)
                    out_ps_by_slot[k] = (out_ps, set())
                out_ps, qdone = out_ps_by_slot[k]

                def mm(qbase, qwid, oh, stop):
                    first = qbase not in qdone
                    qdone.add(qbase)
                    nc.tensor.matmul(
                        out_ps[qbase:qbase + qwid, :], oh[:],
                        rhs_sb[:, tt, 0:D + 1],
                        start=first, stop=stop,
                        skip_group_check=True, tile_position=(0, qbase))

                for tt in range(gsz):
                    t = g0 + tt
                    is_last = (t == t_k - 1)
                    oh, oh2 = ohs[tt]
                    if full:
                        mm(0, P, oh, is_last)
                        continue
                    q = int(q_sched[t])
                    mm(QW * q, QW, oh, is_last and oh2 is None)
                    if oh2 is not None:
                        mm(QW * (q + 1), QW, oh2, is_last)
                if g0 + gsz == t_k:
                    if not full:
                        for q in range(4):
                            if QW * q not in qdone:
                                nc.tensor.matmul(
                                    out_ps[QW * q:QW * (q + 1), :],
                                    wt_sb[:, 0:QW], zrhs_sb[:],
                                    start=True, stop=True,
                                    skip_group_check=True,
                                    tile_position=(0, QW * q))
                    elif not qdone:
                        nc.tensor.matmul(out_ps[:], wt_sb[:], zrhs_sb[:],
                                         start=True, stop=True,
                                         skip_group_check=True)
                    den_sb = midp.tile([P, 1], f32, tag="den")
                    nc.vector.tensor_scalar_max(den_sb[:],
                                                out_ps[:, D:D + 1], 1e-12)
                    rec_sb = midp.tile([P, 1], f32, tag="rec")
                    nc.vector.reciprocal(rec_sb[:], den_sb[:])
                    res_sb = midp.tile([P, D], f32, tag="res")
                    nc.vector.tensor_scalar(
                        res_sb[:], out_ps[:, 0:D], rec_sb[:], None,
                        mybir.AluOpType.mult)
                    nc.scalar.dma_start(out_d[k], res_sb[:])

            pend_e = None      # group awaiting weights (exp already done)
            pend_w = None
            for k, g0, gsz in tasks:
                sl = slots[k]
                if g0 == 0:
                    bl_sb = blp.tile([P, tmax], f32, tag="bl")
                    nc.scalar.dma_start(bl_sb[:, :sl["t_k"]], bl_d[k][:])
                    bl_by_slot[k] = bl_sb
                rhs_sb = rhsp.tile([P, GRP, DC], dt_rhs, tag="rhs")
                nc.sync.dma_start(rhs_sb[:, :gsz],
                                  rhs_d[k][:, g0:g0 + gsz, :])
                ht_sb = htp.tile([P, GRP, D], dt_ht, tag="ht")
                nc.sync.dma_start(ht_sb[:, :gsz],
                                  ht_d[k][:, g0:g0 + gsz, :])
                tanh_cs = []
                for ci, c0 in enumerate(range(0, gsz, CHUNK)):
                    csz = min(CHUNK, gsz - c0)
                    proj_ps = pp.tile([P, CHUNK, D], f32)
                    for tt in range(csz):
                        nc.tensor.matmul(proj_ps[:, tt], wt_sb[:],
                                         ht_sb[:, c0 + tt],
                                         start=True, stop=True)
                    tanh_c = midp.tile([P, CHUNK, D], dt_act, tag="tanh")
                    nc.scalar.activation(
                        tanh_c[:, :csz], proj_ps[:, :csz],
                        mybir.ActivationFunctionType.Tanh, bias=bb_sb[:])
                    tanh_cs.append((c0, csz, tanh_c))
                if pend_w is not None:
                    emit_scatter(pend_w)
                    pend_w = None
                e_list = []
                for ci, (c0, csz, tanh_c) in enumerate(tanh_cs):
                    e_ps_c = ep.tile([QW, P], f32, tag="eps")
                    for tt in range(csz):
                        nc.tensor.matmul(
                            e_ps_c[:, :], wm_sb[:, tt], tanh_c[:, tt],
                            start=(tt == 0), stop=(tt == csz - 1),
                            skip_group_check=True)
                    e_list.append((ci, c0, csz, e_ps_c))
                new_e = emit_exp((k, g0, gsz, rhs_sb, e_list))
                # weights of the previous group are built at the tail of
                # this iteration (their exp is long done); their scatter
                # runs next iteration, filling the PE hole under the tanhs
                pend_w = emit_weights(pend_e) if pend_e is not None else None
                pend_e = new_e
            # drain the pipeline
            if pend_w is not None:
                emit_scatter(pend_w)
            if pend_e is not None:
                emit_scatter(emit_weights(pend_e))
    nc.compile()
    return nc


def kernel(H, batch, W_proj, b_proj, w_score):
    from concourse.bass_utils import run_bass_kernel_spmd

    in_maps, meta = _prep_inputs(H, batch, W_proj, b_proj, w_score, CFG)
    nc = _build_program(meta, CFG)
    res = run_bass_kernel_spmd(nc, in_maps, core_ids=list(range(N_CORES)))
    out = np.empty((G_SEGS, D), np.float32)
    order = meta["order"]
    for c in range(N_CORES):
        slab = res.results[c]["out"]          # [NSLOT, P, D]
        for k in range(NSLOT):
            b = int(order[k * N_CORES + c])
            out[b * P:(b + 1) * P] = slab[k]
    return out.astype(np.float32)


# revision 44
# speedup vs baseline: 1.0512x; 1.0183x over previous
"""AttentiveAggregation (segment softmax-pool) Trainium2 kernel, v2.

Math (per graph g): out_g = sum_v alpha_v H_v,  alpha = softmax_g(e),
  e_v = w_score . tanh(W_proj @ H_v + b_proj).

Key transformations:
 * Global shift: softmax is shift invariant per segment, and
   |e| <= ||w_score||_1 (tanh bounded), so a single global constant
   C = ||w_score||_1 replaces the per-segment max. Then
   out_g = (sum_v a_v [H_v|1])[:D] / (...)[D]  with a_v = exp(e_v - C):
   two segment sums, done as one matmul with a ones-column.
 * Quadrant-windowed segment matmul: segments are grouped into blocks
   of 128 (PSUM accumulator partitions). batch is sorted, so each
   128-node tile spans only a few consecutive segments. The host packs
   nodes so tile t's segments fit [32q(t), 32q(t)+64) for a static
   quadrant schedule q(t); the scatter matmul then uses a [128, 32]
   one-hot lhsT accumulating into PSUM partitions [32q, 32q+32) (PE
   tile_position allows only 32-aligned output bases). Tiles whose
   segments cross into the next quadrant emit one extra matmul (rare).
   The 32-wide one-hot build costs ~1/4 of the full 128-wide one and is
   split between DVE and the otherwise-idle Pool engine.
 * SPMD schedules: one program serves all 8 cores, so q(t) must be
   core-invariant. Blocks are ranked by size; slot k takes ranked
   blocks [8k, 8k+8) (one per core) which share one schedule; per-slot
   tile counts t_k are ragged (saves ~4% DMA vs padding to the max).
 * The projection is computed transposed (h on partitions), so b_proj
   folds into the tanh's per-partition bias for free.
 * Sharding: 8 cores x 16 slots each; cores are fully independent
   (no collectives); host un-permutes the per-core [16,128,128] slabs.
"""

import math

import numpy as np

P = 128                    # partitions / tile node count / D / HS
D = 128
G_SEGS = 16384
SEGS_PER_BLK = 128
NBLK_TOT = G_SEGS // SEGS_PER_BLK   # 128 global blocks
N_CORES = 8
NSLOT = NBLK_TOT // N_CORES         # 16 slots per core
GRP = 24                   # tiles per DMA group
CHUNK = 8                  # tiles per proj-PSUM/tanh chunk
QW = 32                    # quadrant width (PSUM write alignment)

CFG = {
    "ht": "f16",
    "rhs": "f16",
    "act": "f16",
    "oh_dve_mod": 1,       # every Nth one-hot build on DVE, rest on Pool
    "e_op": "pe",          # "pe" | "stt" | "ttr": engine for the score reduce
}

# alpha values are prescaled by exp(PRESCALE_LN) inside the exp bias so the
# smallest per-segment weights stay in f16 normal range; numerator and
# denominator scale together so the final division cancels it exactly.
PRESCALE_LN = 14 * math.log(2.0)

_NP_DT = {"f32": np.float32, "f16": np.float16}


def _np_dt(name):
    if name == "bf16":
        import ml_dtypes
        return ml_dtypes.bfloat16
    return _NP_DT[name]


def _my_dt(name, mybir):
    return {
        "f32": mybir.dt.float32,
        "bf16": mybir.dt.bfloat16,
        "f16": mybir.dt.float16,
    }[name]


def _pack_block(seg_local, q_sched):
    """Greedy pack of one block's nodes onto tiles: tile t may hold nodes
    with local seg in [32*q(t), 32*q(t)+64). Returns list of (pos, take)
    per tile, or None if infeasible."""
    n = len(seg_local)
    pos = 0
    tiles = []
    for t in range(len(q_sched)):
        if pos >= n:
            break
        lo = QW * q_sched[t]
        if seg_local[pos] < lo:
            return None
        hi = np.searchsorted(seg_local, min(lo + 2 * QW, SEGS_PER_BLK),
                             side="left")
        take = int(min(128, hi - pos))
        tiles.append((pos, take))
        pos += take
    if pos < n:
        return None
    return tiles


def _plan(batch):
    """Blocks -> slot plans (quadrant schedules + per-block tilings)."""
    s = np.searchsorted(batch, np.arange(NBLK_TOT + 1, dtype=np.int64)
                        * SEGS_PER_BLK)
    lens = s[1:] - s[:-1]
    order = np.argsort(lens, kind="stable")

    slots = []
    for k in range(NSLOT):
        blks = order[k * N_CORES:(k + 1) * N_CORES]
        maxlen = int(max(1, lens[blks].max()))
        T = int(np.ceil(maxlen / 128)) + 8
        seglists = []
        curves = []
        for b in blks:
            segl = (batch[s[b]:s[b + 1]] - b * SEGS_PER_BLK).astype(np.int64)
            seglists.append(segl)
            sl = segl if len(segl) else np.zeros(1, np.int64)
            idx = np.minimum(np.arange(T) * 128, len(sl) - 1)
            curves.append(sl[idx])
        wmin = np.array(curves).min(axis=0)

        packed = None
        # primary: quadrant schedule from the consensus curve; fallback:
        # all-zero schedule with full-width windows (always feasible)
        for relax in (2, 8, 16, None):
            if relax is None:
                q_sched = np.zeros(T, np.int64)
                full = True
            else:
                w = np.maximum(0, np.minimum(SEGS_PER_BLK - QW, wmin - relax))
                q_sched = np.maximum.accumulate(w // QW).astype(np.int64)
                full = False
            tilings = []
            ok = True
            for segl in seglists:
                tiles = (_pack_block(segl, q_sched) if not full else
                         [(i * 128, int(min(128, len(segl) - i * 128)))
                          for i in range(int(np.ceil(len(segl) / 128)))]
                         or [(0, 0)])
                if tiles is None:
                    ok = False
                    break
                tilings.append(tiles)
            if ok:
                t_k = max(1, max(len(tl) for tl in tilings))
                q_sched = q_sched[:t_k]
                # split flag: any block has a node beyond its tile's quadrant
                split = np.zeros(t_k, bool)
                if not full:
                    for segl, tiles in zip(seglists, tilings):
                        for t, (pos, take) in enumerate(tiles):
                            if take and segl[pos + take - 1] >= \
                                    QW * (q_sched[t] + 1):
                                split[t] = True
                packed = dict(blks=blks, q_sched=q_sched, t_k=t_k,
                              tilings=tilings, split=split, full=full)
                break
        slots.append(packed)
    return s, order, slots


def _prep_inputs(H, batch, W_proj, b_proj, w_score, cfg):
    """Host-side repack. Returns (in_maps, plan_meta)."""
    H = np.ascontiguousarray(H, dtype=np.float32)
    batch = np.asarray(batch).astype(np.int64)
    W_proj = np.asarray(W_proj, dtype=np.float32)
    b_proj = np.asarray(b_proj, dtype=np.float32)
    w_score = np.asarray(w_score, dtype=np.float32)

    c_shift = float(np.abs(w_score).sum())

    s, order, slots = _plan(batch)

    dt_rhs = _np_dt(cfg["rhs"])
    dt_ht = _np_dt(cfg["ht"])
    DC = D + 1                       # H | ones
    H16 = H.astype(dt_rhs)

    in_maps = [dict() for _ in range(N_CORES)]

    for k, sl in enumerate(slots):
        t_k, q_sched = sl["t_k"], sl["q_sched"]
        for c in range(N_CORES):
            b = int(sl["blks"][c])
            tiles = sl["tilings"][c]
            segl = (batch[s[b]:s[b + 1]] - b * SEGS_PER_BLK).astype(np.int64)
            idx = np.full((t_k, 128), -1, np.int64)
            for t, (pos, take) in enumerate(tiles):
                if take:
                    idx[t, :take] = np.arange(s[b] + pos, s[b] + pos + take)
            valid = idx >= 0
            idxc = np.maximum(idx, 0)

            Hg = H16[idxc]
            Hg[~valid] = 0.0
            rhs = np.zeros((t_k, 128, DC), dt_rhs)
            rhs[:, :, :D] = Hg
            rhs[:, :, D] = valid
            bl = np.full((t_k, 128), -1000.0, np.float32)
            for t, (pos, take) in enumerate(tiles):
                if take:
                    bl[t, :take] = segl[pos:pos + take] - QW * q_sched[t]
            m = in_maps[c]
            m[f"rhs{k}"] = np.ascontiguousarray(rhs.transpose(1, 0, 2))
            del rhs
            m[f"bl{k}"] = np.ascontiguousarray(bl.T)
            m[f"ht{k}"] = np.ascontiguousarray(Hg.transpose(2, 0, 1))
            del Hg

    wt = np.ascontiguousarray(W_proj.T.astype(dt_ht))
    wb = np.ascontiguousarray(
        np.broadcast_to(w_score, (P, D)).astype(_np_dt(cfg["act"])))
    # [h, tt, m]: col tt = w_score, other 31 columns zero - each masked
    # e-matmul then zero-fills its whole 32-aligned PSUM region, so the
    # later exp never reads uninitialized PSUM (a HW fault).
    wmask = np.zeros((P, CHUNK, QW), np.float32)
    for tt in range(CHUNK):
        wmask[:, tt, tt] = w_score
    wmask = np.ascontiguousarray(
        wmask.reshape(P, CHUNK * QW).astype(_np_dt(cfg["act"])))
    eye16 = np.zeros((2 * QW, CHUNK), np.float32)
    eye16[0:CHUNK] = np.eye(CHUNK)
    eye16[QW:QW + CHUNK] = np.eye(CHUNK)
    eye16 = np.ascontiguousarray(eye16.astype(_np_dt(cfg["act"])))
    bb = np.ascontiguousarray(b_proj.reshape(P, 1).astype(np.float32))
    iota_lo = np.ascontiguousarray(
        np.broadcast_to(np.arange(QW, dtype=np.float32), (P, QW))
        .astype(dt_rhs))
    iota_hi = np.ascontiguousarray((iota_lo.astype(np.float32) + QW)
                                   .astype(dt_rhs))
    iota_fl = np.ascontiguousarray(
        np.broadcast_to(np.arange(P, dtype=np.float32), (P, P))
        .astype(dt_rhs))
    for c in range(N_CORES):
        in_maps[c]["wt"] = wt
        in_maps[c]["wb"] = wb
        in_maps[c]["wmask"] = wmask
        in_maps[c]["eye16"] = eye16
        in_maps[c]["iota_lo"] = iota_lo
        in_maps[c]["iota_hi"] = iota_hi
        in_maps[c]["iota_fl"] = iota_fl
        in_maps[c]["bb"] = bb

    meta = dict(slots=[dict(t_k=sl["t_k"], q_sched=sl["q_sched"].tolist(),
                            split=sl["split"].tolist(), full=sl["full"])
                       for sl in slots],
                order=order, c_shift=c_shift)
    return in_maps, meta


def _build_program(meta, cfg):
    import concourse.bacc as bacc
    import concourse.mybir as mybir
    import concourse.tile as tile

    f32 = mybir.dt.float32
    dt_ht = _my_dt(cfg["ht"], mybir)
    dt_rhs = _my_dt(cfg["rhs"], mybir)
    dt_act = _my_dt(cfg["act"], mybir)
    c_shift = meta["c_shift"]
    slots = meta["slots"]
    DC = D + 1
    oh_mod = int(cfg["oh_dve_mod"])

    nc = bacc.Bacc(None, target_bir_lowering=False)

    rhs_d, ht_d = [], []
    for k, sl in enumerate(slots):
        rhs_d.append(nc.dram_tensor(f"rhs{k}", [P, sl["t_k"], DC], dt_rhs,
                                    kind="ExternalInput"))
        ht_d.append(nc.dram_tensor(f"ht{k}", [P, sl["t_k"], D], dt_ht,
                                   kind="ExternalInput"))
    bl_d = [nc.dram_tensor(f"bl{k}", [P, sl["t_k"]], f32,
                           kind="ExternalInput")
            for k, sl in enumerate(slots)]
    wt_d = nc.dram_tensor("wt", [P, D], dt_ht, kind="ExternalInput")
    wb_d = nc.dram_tensor("wb", [P, D], dt_act, kind="ExternalInput")
    wm_d = nc.dram_tensor("wmask", [P, CHUNK * QW], dt_act,
                          kind="ExternalInput")
    ey_d = nc.dram_tensor("eye16", [2 * QW, CHUNK], dt_act,
                          kind="ExternalInput")
    ilo_d = nc.dram_tensor("iota_lo", [P, QW], dt_rhs, kind="ExternalInput")
    ihi_d = nc.dram_tensor("iota_hi", [P, QW], dt_rhs, kind="ExternalInput")
    ifl_d = nc.dram_tensor("iota_fl", [P, P], dt_rhs, kind="ExternalInput")
    bb_d = nc.dram_tensor("bb", [P, 1], f32, kind="ExternalInput")
    out_d = nc.dram_tensor("out", [NSLOT, P, D], f32, kind="ExternalOutput")

    with tile.TileContext(nc) as tc:
        with (
            tc.tile_pool(name="const", bufs=1) as constp,
            tc.tile_pool(name="rhsp", bufs=9) as rhsp,
            tc.tile_pool(name="htp", bufs=8) as htp,
            tc.tile_pool(name="midp", bufs=8) as midp,
            tc.tile_pool(name="ohp", bufs=60) as ohp,
            tc.tile_pool(name="blp", bufs=2) as blp,
            tc.tile_pool(name="pp", bufs=2, space="PSUM") as pp,
            tc.tile_pool(name="op", bufs=1, space="PSUM") as op,
            tc.tile_pool(name="ep", bufs=2, space="PSUM") as ep,
            tc.tile_pool(name="tp", bufs=1, space="PSUM") as tp,
        ):
            wt_sb = constp.tile([P, D], dt_ht)
            nc.scalar.dma_start(wt_sb[:], wt_d[:])
            wb_sb = constp.tile([P, D], dt_act)
            nc.scalar.dma_start(wb_sb[:], wb_d[:])
            wm_sb = constp.tile([P, CHUNK, QW], dt_act)
            nc.scalar.dma_start(wm_sb[:], wm_d[:].rearrange(
                "p (t m) -> p t m", t=CHUNK))
            ey_sb = constp.tile([2 * QW, CHUNK], dt_act)
            nc.scalar.dma_start(ey_sb[:], ey_d[:])
            ilo_sb = constp.tile([P, QW], dt_rhs)
            nc.scalar.dma_start(ilo_sb[:], ilo_d[:])
            ihi_sb = constp.tile([P, QW], dt_rhs)
            nc.scalar.dma_start(ihi_sb[:], ihi_d[:])
            ifl_sb = constp.tile([P, P], dt_rhs)
            nc.scalar.dma_start(ifl_sb[:], ifl_d[:])
            zrhs_sb = constp.tile([P, D + 1], dt_rhs)
            nc.gpsimd.memset(zrhs_sb[:], 0.0)
            negc_sb = constp.tile([P, 1], f32)
            nc.gpsimd.memset(negc_sb[:], -float(c_shift) + PRESCALE_LN)
            bb_sb = constp.tile([P, 1], f32)
            nc.scalar.dma_start(bb_sb[:], bb_d[:])

            tmax = max(sl["t_k"] for sl in slots)
            tasks = []
            # largest slots first: the pipeline drain then runs on the
            # smallest slot, shortening the tail
            for k in sorted(range(NSLOT), key=lambda k: -slots[k]["t_k"]):
                for g0 in range(0, slots[k]["t_k"], GRP):
                    tasks.append((k, g0, min(GRP, slots[k]["t_k"] - g0)))

            out_ps_by_slot = {}
            bl_by_slot = {}
            oh_ctr = 0

            def emit_exp_chunk(e_ps_c):
                aT = midp.tile([QW, P], dt_act, tag="aT")
                nc.scalar.activation(
                    aT[:QW], e_ps_c[:QW],
                    mybir.ActivationFunctionType.Exp, bias=negc_sb[:QW])
                return aT

            def emit_weights(pend):
                """transpose+copy of a, then the one-hot builds (1 group
                ahead of the scatter matmuls). Returns oh tile list."""
                nonlocal oh_ctr
                k, g0, gsz, rhs_sb, aTs = pend
                sl = slots[k]
                a_ps = tp.tile([P, GRP], dt_act)
                for ci, c0, csz, aT in aTs:
                    nc.tensor.transpose(a_ps[:, c0:c0 + csz], aT[:csz],
                                        ey_sb[:csz, :csz])
                a16 = midp.tile([P, GRP], f32, tag="a16")
                nc.vector.tensor_copy(a16[:, :gsz], a_ps[:, :gsz])
                bl_sb = bl_by_slot[k]
                ohs = []
                for tt in range(gsz):
                    t = g0 + tt
                    bl_ap = bl_sb[:, t:t + 1]
                    a_ap = a16[:, tt:tt + 1]
                    wide = QW * 4 if sl["full"] else QW
                    iota = ifl_sb if sl["full"] else ilo_sb
                    oh = ohp.tile([P, wide], dt_rhs,
                                  tag="ohf" if sl["full"] else "oh")
                    eng = nc.vector if oh_ctr % oh_mod == 0 else nc.gpsimd
                    oh_ctr += 1
                    eng.tensor_scalar(
                        oh[:], iota[:], bl_ap, a_ap,
                        mybir.AluOpType.is_equal, mybir.AluOpType.mult)
                    oh2 = None
                    q = int(sl["q_sched"][t]) if not sl["full"] else 0
                    if not sl["full"] and bool(sl["split"][t]) and q < 3:
                        oh2 = ohp.tile([P, QW], dt_rhs, tag="oh2")
                        nc.vector.tensor_scalar(
                            oh2[:], ihi_sb[:], bl_ap, a_ap,
                            mybir.AluOpType.is_equal, mybir.AluOpType.mult)
                    ohs.append((oh, oh2))
                return (k, g0, gsz, rhs_sb, ohs)

            def emit_scatter(pend):
                k, g0, gsz, rhs_sb, ohs = pend
                sl = slots[k]
                t_k, q_sched, full = sl["t_k"], sl["q_sched"], sl["full"]
                if g0 == 0:
                    out_ps = op.tile([P, D + 1], f32)
                    out_ps_by_slot[k] = (out_ps, set())
                out_ps, qdone = out_ps_by_slot[k]

                def mm(qbase, qwid, oh, stop):
                    first = qbase not in qdone
                    qdone.add(qbase)
                    nc.tensor.matmul(
                        out_ps[qbase:qbase + qwid, :], oh[:],
                        rhs_sb[:, tt, 0:D + 1],
                        start=first, stop=stop,
                        skip_group_check=True, tile_position=(0, qbase))

                for tt in range(gsz):
                    t = g0 + tt
                    is_last = (t == t_k - 1)
                    oh, oh2 = ohs[tt]
                    if full:
                        mm(0, P, oh, is_last)
                        continue
                    q = int(q_sched[t])
                    mm(QW * q, QW, oh, is_last and oh2 is None)
                    if oh2 is not None:
                        mm(QW * (q + 1), QW, oh2, is_last)
                if g0 + gsz == t_k:
                    if not full:
                        for q in range(4):
                            if QW * q not in qdone:
                                nc.tensor.matmul(
                                    out_ps[QW * q:QW * (q + 1), :],
                                    wt_sb[:, 0:QW], zrhs_sb[:],
                                    start=True, stop=True,
                                    skip_group_check=True,
                                    tile_position=(0, QW * q))
                    elif not qdone:
                        nc.tensor.matmul(out_ps[:], wt_sb[:], zrhs_sb[:],
                                         start=True, stop=True,
                                         skip_group_check=True)
                    den_sb = midp.tile([P, 1], f32, tag="den")
                    nc.vector.tensor_scalar_max(den_sb[:],
                                                out_ps[:, D:D + 1], 1e-12)
                    rec_sb = midp.tile([P, 1], f32, tag="rec")
                    nc.vector.reciprocal(rec_sb[:], den_sb[:])
                    res_sb = midp.tile([P, D], f32, tag="res")
                    nc.vector.tensor_scalar(
                        res_sb[:], out_ps[:, 0:D], rec_sb[:], None,
                        mybir.AluOpType.mult)
                    nc.scalar.dma_start(out_d[k], res_sb[:])

            pend_e = None      # group awaiting weights (exp already done)
            pend_w = None
            pend_w2 = None
            for k, g0, gsz in tasks:
                sl = slots[k]
                if g0 == 0:
                    bl_sb = blp.tile([P, tmax], f32, tag="bl")
                    nc.scalar.dma_start(bl_sb[:, :sl["t_k"]], bl_d[k][:])
                    bl_by_slot[k] = bl_sb
                rhs_sb = rhsp.tile([P, GRP, DC], dt_rhs, tag="rhs")
                nc.sync.dma_start(rhs_sb[:, :gsz],
                                  rhs_d[k][:, g0:g0 + gsz, :])
                ht_sb = htp.tile([P, GRP, D], dt_ht, tag="ht")
                nc.sync.dma_start(ht_sb[:, :gsz],
                                  ht_d[k][:, g0:g0 + gsz, :])
                tanh_cs = []
                for ci, c0 in enumerate(range(0, gsz, CHUNK)):
                    csz = min(CHUNK, gsz - c0)
                    proj_ps = pp.tile([P, CHUNK, D], f32)
                    for tt in range(csz):
                        nc.tensor.matmul(proj_ps[:, tt], wt_sb[:],
                                         ht_sb[:, c0 + tt],
                                         start=True, stop=True)
                    tanh_c = midp.tile([P, CHUNK, D], dt_act, tag="tanh")
                    nc.scalar.activation(
                        tanh_c[:, :csz], proj_ps[:, :csz],
                        mybir.ActivationFunctionType.Tanh, bias=bb_sb[:])
                    tanh_cs.append((c0, csz, tanh_c))
                if pend_w is not None:
                    emit_scatter(pend_w)
                    pend_w = None
                aTs = []
                for ci, (c0, csz, tanh_c) in enumerate(tanh_cs):
                    e_ps_c = ep.tile([QW, P], f32, tag="eps")
                    for tt in range(csz):
                        nc.tensor.matmul(
                            e_ps_c[:, :], wm_sb[:, tt], tanh_c[:, tt],
                            start=(tt == 0), stop=(tt == csz - 1),
                            skip_group_check=True)
                    aTs.append((ci, c0, csz, emit_exp_chunk(e_ps_c)))
                new_e = (k, g0, gsz, rhs_sb, aTs)
                # weights of the previous group are built at the tail of
                # this iteration (their exp is long done); their scatter
                # runs next iteration, filling the PE hole under the tanhs
                pend_w = emit_weights(pend_e) if pend_e is not None else None
                pend_e = new_e
            # drain the pipeline
            for pw in (pend_w2, pend_w):
                if pw is not None:
                    emit_scatter(pw)
            if pend_e is not None:
                emit_scatter(emit_weights(pend_e))
    nc.compile()
    return nc


def kernel(H, batch, W_proj, b_proj, w_score):
    from concourse.bass_utils import run_bass_kernel_spmd

    in_maps, meta = _prep_inputs(H, batch, W_proj, b_proj, w_score, CFG)
    nc = _build_program(meta, CFG)
    res = run_bass_kernel_spmd(nc, in_maps, core_ids=list(range(N_CORES)))
    out = np.empty((G_SEGS, D), np.float32)
    order = meta["order"]
    for c in range(N_CORES):
        slab = res.results[c]["out"]          # [NSLOT, P, D]
        for k in range(NSLOT):
            b = int(order[k * N_CORES + c])
            out[b * P:(b + 1) * P] = slab[k]
    return out.astype(np.float32)


# revision 47
# speedup vs baseline: 1.0800x; 1.0274x over previous
"""AttentiveAggregation (segment softmax-pool) Trainium2 kernel, v2.

Math (per graph g): out_g = sum_v alpha_v H_v,  alpha = softmax_g(e),
  e_v = w_score . tanh(W_proj @ H_v + b_proj).

Key transformations:
 * Global shift: softmax is shift invariant per segment, and
   |e| <= ||w_score||_1 (tanh bounded), so a single global constant
   C = ||w_score||_1 replaces the per-segment max. Then
   out_g = (sum_v a_v [H_v|1])[:D] / (...)[D]  with a_v = exp(e_v - C):
   two segment sums, done as one matmul with a ones-column.
 * Quadrant-windowed segment matmul: segments are grouped into blocks
   of 128 (PSUM accumulator partitions). batch is sorted, so each
   128-node tile spans only a few consecutive segments. The host packs
   nodes so tile t's segments fit [32q(t), 32q(t)+64) for a static
   quadrant schedule q(t); the scatter matmul then uses a [128, 32]
   one-hot lhsT accumulating into PSUM partitions [32q, 32q+32) (PE
   tile_position allows only 32-aligned output bases). Tiles whose
   segments cross into the next quadrant emit one extra matmul (rare).
   The 32-wide one-hot build costs ~1/4 of the full 128-wide one and is
   split between DVE and the otherwise-idle Pool engine.
 * SPMD schedules: one program serves all 8 cores, so q(t) must be
   core-invariant. Blocks are ranked by size; slot k takes ranked
   blocks [8k, 8k+8) (one per core) which share one schedule; per-slot
   tile counts t_k are ragged (saves ~4% DMA vs padding to the max).
 * The projection is computed transposed (h on partitions), so b_proj
   folds into the tanh's per-partition bias for free.
 * Sharding: 8 cores x 16 slots each; cores are fully independent
   (no collectives); host un-permutes the per-core [16,128,128] slabs.
"""

import math

import numpy as np

P = 128                    # partitions / tile node count / D / HS
D = 128
G_SEGS = 16384
SEGS_PER_BLK = 128
NBLK_TOT = G_SEGS // SEGS_PER_BLK   # 128 global blocks
N_CORES = 8
NSLOT = NBLK_TOT // N_CORES         # 16 slots per core
GRP = 28                   # tiles per DMA group
CHUNK = 8                  # tiles per proj-PSUM/tanh chunk
QW = 32                    # quadrant width (PSUM write alignment)

CFG = {
    "ht": "f16",
    "rhs": "f16",
    "act": "f16",
    "oh_dve_mod": 1,       # every Nth one-hot build on DVE, rest on Pool
    "e_op": "pe",          # "pe" | "stt" | "ttr": engine for the score reduce
}

# alpha values are prescaled by exp(PRESCALE_LN) inside the exp bias so the
# smallest per-segment weights stay in f16 normal range; numerator and
# denominator scale together so the final division cancels it exactly.
PRESCALE_LN = 14 * math.log(2.0)

_NP_DT = {"f32": np.float32, "f16": np.float16}


def _np_dt(name):
    if name == "bf16":
        import ml_dtypes
        return ml_dtypes.bfloat16
    return _NP_DT[name]


def _my_dt(name, mybir):
    return {
        "f32": mybir.dt.float32,
        "bf16": mybir.dt.bfloat16,
        "f16": mybir.dt.float16,
    }[name]


def _pack_block(seg_local, q_sched):
    """Greedy pack of one block's nodes onto tiles: tile t may hold nodes
    with local seg in [32*q(t), 32*q(t)+64). Returns list of (pos, take)
    per tile, or None if infeasible."""
    n = len(seg_local)
    pos = 0
    tiles = []
    for t in range(len(q_sched)):
        if pos >= n:
            break
        lo = QW * q_sched[t]
        if seg_local[pos] < lo:
            return None
        hi = np.searchsorted(seg_local, min(lo + 2 * QW, SEGS_PER_BLK),
                             side="left")
        take = int(min(128, hi - pos))
        tiles.append((pos, take))
        pos += take
    if pos < n:
        return None
    return tiles


def _plan(batch):
    """Blocks -> slot plans (quadrant schedules + per-block tilings)."""
    s = np.searchsorted(batch, np.arange(NBLK_TOT + 1, dtype=np.int64)
                        * SEGS_PER_BLK)
    lens = s[1:] - s[:-1]
    order = np.argsort(lens, kind="stable")

    slots = []
    for k in range(NSLOT):
        blks = order[k * N_CORES:(k + 1) * N_CORES]
        maxlen = int(max(1, lens[blks].max()))
        T = int(np.ceil(maxlen / 128)) + 8
        seglists = []
        curves = []
        for b in blks:
            segl = (batch[s[b]:s[b + 1]] - b * SEGS_PER_BLK).astype(np.int64)
            seglists.append(segl)
            sl = segl if len(segl) else np.zeros(1, np.int64)
            idx = np.minimum(np.arange(T) * 128, len(sl) - 1)
            curves.append(sl[idx])
        wmin = np.array(curves).min(axis=0)

        packed = None
        # primary: quadrant schedule from the consensus curve; fallback:
        # all-zero schedule with full-width windows (always feasible)
        for relax in (2, 8, 16, None):
            if relax is None:
                q_sched = np.zeros(T, np.int64)
                full = True
            else:
                w = np.maximum(0, np.minimum(SEGS_PER_BLK - QW, wmin - relax))
                q_sched = np.maximum.accumulate(w // QW).astype(np.int64)
                full = False
            tilings = []
            ok = True
            for segl in seglists:
                tiles = (_pack_block(segl, q_sched) if not full else
                         [(i * 128, int(min(128, len(segl) - i * 128)))
                          for i in range(int(np.ceil(len(segl) / 128)))]
                         or [(0, 0)])
                if tiles is None:
                    ok = False
                    break
                tilings.append(tiles)
            if ok:
                t_k = max(1, max(len(tl) for tl in tilings))
                q_sched = q_sched[:t_k]
                # split flag: any block has a node beyond its tile's quadrant
                split = np.zeros(t_k, bool)
                if not full:
                    for segl, tiles in zip(seglists, tilings):
                        for t, (pos, take) in enumerate(tiles):
                            if take and segl[pos + take - 1] >= \
                                    QW * (q_sched[t] + 1):
                                split[t] = True
                packed = dict(blks=blks, q_sched=q_sched, t_k=t_k,
                              tilings=tilings, split=split, full=full)
                break
        slots.append(packed)
    return s, order, slots


def _prep_inputs(H, batch, W_proj, b_proj, w_score, cfg):
    """Host-side repack. Returns (in_maps, plan_meta)."""
    H = np.ascontiguousarray(H, dtype=np.float32)
    batch = np.asarray(batch).astype(np.int64)
    W_proj = np.asarray(W_proj, dtype=np.float32)
    b_proj = np.asarray(b_proj, dtype=np.float32)
    w_score = np.asarray(w_score, dtype=np.float32)

    c_shift = float(np.abs(w_score).sum())

    s, order, slots = _plan(batch)

    dt_rhs = _np_dt(cfg["rhs"])
    dt_ht = _np_dt(cfg["ht"])
    DC = D + 1                       # H | ones
    H16 = H.astype(dt_rhs)

    in_maps = [dict() for _ in range(N_CORES)]

    for k, sl in enumerate(slots):
        t_k, q_sched = sl["t_k"], sl["q_sched"]
        for c in range(N_CORES):
            b = int(sl["blks"][c])
            tiles = sl["tilings"][c]
            segl = (batch[s[b]:s[b + 1]] - b * SEGS_PER_BLK).astype(np.int64)
            idx = np.full((t_k, 128), -1, np.int64)
            for t, (pos, take) in enumerate(tiles):
                if take:
                    idx[t, :take] = np.arange(s[b] + pos, s[b] + pos + take)
            valid = idx >= 0
            idxc = np.maximum(idx, 0)

            Hg = H16[idxc]
            Hg[~valid] = 0.0
            rhs = np.zeros((t_k, 128, DC), dt_rhs)
            rhs[:, :, :D] = Hg
            rhs[:, :, D] = valid
            bl = np.full((t_k, 128), -1000.0, np.float32)
            for t, (pos, take) in enumerate(tiles):
                if take:
                    bl[t, :take] = segl[pos:pos + take] - QW * q_sched[t]
            m = in_maps[c]
            m[f"rhs{k}"] = np.ascontiguousarray(rhs.transpose(1, 0, 2))
            del rhs
            m[f"bl{k}"] = np.ascontiguousarray(bl.T)
            m[f"ht{k}"] = np.ascontiguousarray(Hg.transpose(2, 0, 1))
            del Hg

    wt = np.ascontiguousarray(W_proj.T.astype(dt_ht))
    wb = np.ascontiguousarray(
        np.broadcast_to(w_score, (P, D)).astype(_np_dt(cfg["act"])))
    # [h, tt, m]: col tt = w_score, other 31 columns zero - each masked
    # e-matmul then zero-fills its whole 32-aligned PSUM region, so the
    # later exp never reads uninitialized PSUM (a HW fault).
    wmask = np.zeros((P, CHUNK, QW), np.float32)
    for tt in range(CHUNK):
        wmask[:, tt, tt] = w_score
    wmask = np.ascontiguousarray(
        wmask.reshape(P, CHUNK * QW).astype(_np_dt(cfg["act"])))
    eye16 = np.zeros((2 * QW, CHUNK), np.float32)
    eye16[0:CHUNK] = np.eye(CHUNK)
    eye16[QW:QW + CHUNK] = np.eye(CHUNK)
    eye16 = np.ascontiguousarray(eye16.astype(_np_dt(cfg["act"])))
    bb = np.ascontiguousarray(b_proj.reshape(P, 1).astype(np.float32))
    iota_lo = np.ascontiguousarray(
        np.broadcast_to(np.arange(QW, dtype=np.float32), (P, QW))
        .astype(dt_rhs))
    iota_hi = np.ascontiguousarray((iota_lo.astype(np.float32) + QW)
                                   .astype(dt_rhs))
    iota_fl = np.ascontiguousarray(
        np.broadcast_to(np.arange(P, dtype=np.float32), (P, P))
        .astype(dt_rhs))
    for c in range(N_CORES):
        in_maps[c]["wt"] = wt
        in_maps[c]["wb"] = wb
        in_maps[c]["wmask"] = wmask
        in_maps[c]["eye16"] = eye16
        in_maps[c]["iota_lo"] = iota_lo
        in_maps[c]["iota_hi"] = iota_hi
        in_maps[c]["iota_fl"] = iota_fl
        in_maps[c]["bb"] = bb

    meta = dict(slots=[dict(t_k=sl["t_k"], q_sched=sl["q_sched"].tolist(),
                            split=sl["split"].tolist(), full=sl["full"])
                       for sl in slots],
                order=order, c_shift=c_shift)
    return in_maps, meta


def _build_program(meta, cfg):
    import concourse.bacc as bacc
    import concourse.mybir as mybir
    import concourse.tile as tile

    f32 = mybir.dt.float32
    dt_ht = _my_dt(cfg["ht"], mybir)
    dt_rhs = _my_dt(cfg["rhs"], mybir)
    dt_act = _my_dt(cfg["act"], mybir)
    c_shift = meta["c_shift"]
    slots = meta["slots"]
    DC = D + 1
    oh_mod = int(cfg["oh_dve_mod"])

    nc = bacc.Bacc(None, target_bir_lowering=False)

    rhs_d, ht_d = [], []
    for k, sl in enumerate(slots):
        rhs_d.append(nc.dram_tensor(f"rhs{k}", [P, sl["t_k"], DC], dt_rhs,
                                    kind="ExternalInput"))
        ht_d.append(nc.dram_tensor(f"ht{k}", [P, sl["t_k"], D], dt_ht,
                                   kind="ExternalInput"))
    bl_d = [nc.dram_tensor(f"bl{k}", [P, sl["t_k"]], f32,
                           kind="ExternalInput")
            for k, sl in enumerate(slots)]
    wt_d = nc.dram_tensor("wt", [P, D], dt_ht, kind="ExternalInput")
    wb_d = nc.dram_tensor("wb", [P, D], dt_act, kind="ExternalInput")
    wm_d = nc.dram_tensor("wmask", [P, CHUNK * QW], dt_act,
                          kind="ExternalInput")
    ey_d = nc.dram_tensor("eye16", [2 * QW, CHUNK], dt_act,
                          kind="ExternalInput")
    ilo_d = nc.dram_tensor("iota_lo", [P, QW], dt_rhs, kind="ExternalInput")
    ihi_d = nc.dram_tensor("iota_hi", [P, QW], dt_rhs, kind="ExternalInput")
    ifl_d = nc.dram_tensor("iota_fl", [P, P], dt_rhs, kind="ExternalInput")
    bb_d = nc.dram_tensor("bb", [P, 1], f32, kind="ExternalInput")
    out_d = nc.dram_tensor("out", [NSLOT, P, D], f32, kind="ExternalOutput")

    with tile.TileContext(nc) as tc:
        with (
            tc.tile_pool(name="const", bufs=1) as constp,
            tc.tile_pool(name="rhsp", bufs=9) as rhsp,
            tc.tile_pool(name="htp", bufs=8) as htp,
            tc.tile_pool(name="midp", bufs=8) as midp,
            tc.tile_pool(name="ohp", bufs=60) as ohp,
            tc.tile_pool(name="blp", bufs=2) as blp,
            tc.tile_pool(name="pp", bufs=2, space="PSUM") as pp,
            tc.tile_pool(name="op", bufs=1, space="PSUM") as op,
            tc.tile_pool(name="ep", bufs=2, space="PSUM") as ep,
            tc.tile_pool(name="tp", bufs=1, space="PSUM") as tp,
        ):
            wt_sb = constp.tile([P, D], dt_ht)
            nc.scalar.dma_start(wt_sb[:], wt_d[:])
            wb_sb = constp.tile([P, D], dt_act)
            nc.scalar.dma_start(wb_sb[:], wb_d[:])
            wm_sb = constp.tile([P, CHUNK, QW], dt_act)
            nc.scalar.dma_start(wm_sb[:], wm_d[:].rearrange(
                "p (t m) -> p t m", t=CHUNK))
            ey_sb = constp.tile([2 * QW, CHUNK], dt_act)
            nc.scalar.dma_start(ey_sb[:], ey_d[:])
            ilo_sb = constp.tile([P, QW], dt_rhs)
            nc.scalar.dma_start(ilo_sb[:], ilo_d[:])
            ihi_sb = constp.tile([P, QW], dt_rhs)
            nc.scalar.dma_start(ihi_sb[:], ihi_d[:])
            ifl_sb = constp.tile([P, P], dt_rhs)
            nc.scalar.dma_start(ifl_sb[:], ifl_d[:])
            zrhs_sb = constp.tile([P, D + 1], dt_rhs)
            nc.gpsimd.memset(zrhs_sb[:], 0.0)
            negc_sb = constp.tile([P, 1], f32)
            nc.gpsimd.memset(negc_sb[:], -float(c_shift) + PRESCALE_LN)
            bb_sb = constp.tile([P, 1], f32)
            nc.scalar.dma_start(bb_sb[:], bb_d[:])

            tmax = max(sl["t_k"] for sl in slots)
            tasks = []
            # largest slots first: the pipeline drain then runs on the
            # smallest slot, shortening the tail
            for k in sorted(range(NSLOT), key=lambda k: -slots[k]["t_k"]):
                for g0 in range(0, slots[k]["t_k"], GRP):
                    tasks.append((k, g0, min(GRP, slots[k]["t_k"] - g0)))

            out_ps_by_slot = {}
            bl_by_slot = {}
            oh_ctr = 0

            def emit_exp_chunk(e_ps_c):
                aT = midp.tile([QW, P], dt_act, tag="aT")
                nc.scalar.activation(
                    aT[:QW], e_ps_c[:QW],
                    mybir.ActivationFunctionType.Exp, bias=negc_sb[:QW])
                return aT

            def emit_weights(pend):
                """transpose+copy of a, then the one-hot builds (1 group
                ahead of the scatter matmuls). Returns oh tile list."""
                nonlocal oh_ctr
                k, g0, gsz, rhs_sb, aTs = pend
                sl = slots[k]
                a_ps = tp.tile([P, GRP], dt_act)
                for ci, c0, csz, aT in aTs:
                    nc.tensor.transpose(a_ps[:, c0:c0 + csz], aT[:csz],
                                        ey_sb[:csz, :csz])
                a16 = midp.tile([P, GRP], f32, tag="a16")
                nc.vector.tensor_copy(a16[:, :gsz], a_ps[:, :gsz])
                bl_sb = bl_by_slot[k]
                ohs = []
                for tt in range(gsz):
                    t = g0 + tt
                    bl_ap = bl_sb[:, t:t + 1]
                    a_ap = a16[:, tt:tt + 1]
                    wide = QW * 4 if sl["full"] else QW
                    iota = ifl_sb if sl["full"] else ilo_sb
                    oh = ohp.tile([P, wide], dt_rhs,
                                  tag="ohf" if sl["full"] else "oh")
                    eng = nc.vector if oh_ctr % oh_mod == 0 else nc.gpsimd
                    oh_ctr += 1
                    eng.tensor_scalar(
                        oh[:], iota[:], bl_ap, a_ap,
                        mybir.AluOpType.is_equal, mybir.AluOpType.mult)
                    oh2 = None
                    q = int(sl["q_sched"][t]) if not sl["full"] else 0
                    if not sl["full"] and bool(sl["split"][t]) and q < 3:
                        oh2 = ohp.tile([P, QW], dt_rhs, tag="oh2")
                        nc.vector.tensor_scalar(
                            oh2[:], ihi_sb[:], bl_ap, a_ap,
                            mybir.AluOpType.is_equal, mybir.AluOpType.mult)
                    ohs.append((oh, oh2))
                return (k, g0, gsz, rhs_sb, ohs)

            def emit_scatter(pend):
                k, g0, gsz, rhs_sb, ohs = pend
                sl = slots[k]
                t_k, q_sched, full = sl["t_k"], sl["q_sched"], sl["full"]
                if g0 == 0:
                    out_ps = op.tile([P, D + 1], f32)
                    out_ps_by_slot[k] = (out_ps, set())
                out_ps, qdone = out_ps_by_slot[k]

                def mm(qbase, qwid, oh, stop):
                    first = qbase not in qdone
                    qdone.add(qbase)
                    nc.tensor.matmul(
                        out_ps[qbase:qbase + qwid, :], oh[:],
                        rhs_sb[:, tt, 0:D + 1],
                        start=first, stop=stop,
                        skip_group_check=True, tile_position=(0, qbase))

                for tt in range(gsz):
                    t = g0 + tt
                    is_last = (t == t_k - 1)
                    oh, oh2 = ohs[tt]
                    if full:
                        mm(0, P, oh, is_last)
                        continue
                    q = int(q_sched[t])
                    mm(QW * q, QW, oh, is_last and oh2 is None)
                    if oh2 is not None:
                        mm(QW * (q + 1), QW, oh2, is_last)
                if g0 + gsz == t_k:
                    if not full:
                        for q in range(4):
                            if QW * q not in qdone:
                                nc.tensor.matmul(
                                    out_ps[QW * q:QW * (q + 1), :],
                                    wt_sb[:, 0:QW], zrhs_sb[:],
                                    start=True, stop=True,
                                    skip_group_check=True,
                                    tile_position=(0, QW * q))
                    elif not qdone:
                        nc.tensor.matmul(out_ps[:], wt_sb[:], zrhs_sb[:],
                                         start=True, stop=True,
                                         skip_group_check=True)
                    den_sb = midp.tile([P, 1], f32, tag="den")
                    nc.vector.tensor_scalar_max(den_sb[:],
                                                out_ps[:, D:D + 1], 1e-12)
                    rec_sb = midp.tile([P, 1], f32, tag="rec")
                    nc.vector.reciprocal(rec_sb[:], den_sb[:])
                    res_sb = midp.tile([P, D], f32, tag="res")
                    nc.vector.tensor_scalar(
                        res_sb[:], out_ps[:, 0:D], rec_sb[:], None,
                        mybir.AluOpType.mult)
                    nc.scalar.dma_start(out_d[k], res_sb[:])

            pend_e = None      # group awaiting weights (exp already done)
            pend_w = None
            pend_w2 = None
            for k, g0, gsz in tasks:
                sl = slots[k]
                if g0 == 0:
                    bl_sb = blp.tile([P, tmax], f32, tag="bl")
                    nc.scalar.dma_start(bl_sb[:, :sl["t_k"]], bl_d[k][:])
                    bl_by_slot[k] = bl_sb
                rhs_sb = rhsp.tile([P, GRP, DC], dt_rhs, tag="rhs")
                nc.sync.dma_start(rhs_sb[:, :gsz],
                                  rhs_d[k][:, g0:g0 + gsz, :])
                ht_sb = htp.tile([P, GRP, D], dt_ht, tag="ht")
                nc.sync.dma_start(ht_sb[:, :gsz],
                                  ht_d[k][:, g0:g0 + gsz, :])
                tanh_cs = []
                for ci, c0 in enumerate(range(0, gsz, CHUNK)):
                    csz = min(CHUNK, gsz - c0)
                    proj_ps = pp.tile([P, CHUNK, D], f32)
                    for tt in range(csz):
                        nc.tensor.matmul(proj_ps[:, tt], wt_sb[:],
                                         ht_sb[:, c0 + tt],
                                         start=True, stop=True)
                    tanh_c = midp.tile([P, CHUNK, D], dt_act, tag="tanh")
                    nc.scalar.activation(
                        tanh_c[:, :csz], proj_ps[:, :csz],
                        mybir.ActivationFunctionType.Tanh, bias=bb_sb[:])
                    tanh_cs.append((c0, csz, tanh_c))
                if pend_w is not None:
                    emit_scatter(pend_w)
                    pend_w = None
                aTs = []
                for ci, (c0, csz, tanh_c) in enumerate(tanh_cs):
                    e_ps_c = ep.tile([QW, P], f32, tag="eps")
                    for tt in range(csz):
                        nc.tensor.matmul(
                            e_ps_c[:, :], wm_sb[:, tt], tanh_c[:, tt],
                            start=(tt == 0), stop=(tt == csz - 1),
                            skip_group_check=True)
                    aTs.append((ci, c0, csz, emit_exp_chunk(e_ps_c)))
                new_e = (k, g0, gsz, rhs_sb, aTs)
                # weights of the previous group are built at the tail of
                # this iteration (their exp is long done); their scatter
                # runs next iteration, filling the PE hole under the tanhs
                pend_w = emit_weights(pend_e) if pend_e is not None else None
                pend_e = new_e
            # drain the pipeline
            for pw in (pend_w2, pend_w):
                if pw is not None:
                    emit_scatter(pw)
            if pend_e is not None:
                emit_scatter(emit_weights(pend_e))
    nc.compile()
    return nc


def kernel(H, batch, W_proj, b_proj, w_score):
    from concourse.bass_utils import run_bass_kernel_spmd

    in_maps, meta = _prep_inputs(H, batch, W_proj, b_proj, w_score, CFG)
    nc = _build_program(meta, CFG)
    res = run_bass_kernel_spmd(nc, in_maps, core_ids=list(range(N_CORES)))
    out = np.empty((G_SEGS, D), np.float32)
    order = meta["order"]
    for c in range(N_CORES):
        slab = res.results[c]["out"]          # [NSLOT, P, D]
        for k in range(NSLOT):
            b = int(order[k * N_CORES + c])
            out[b * P:(b + 1) * P] = slab[k]
    return out.astype(np.float32)


# revision 54
# speedup vs baseline: 4.0533x; 3.7529x over previous
"""AttentiveAggregation (segment softmax-pool) Trainium2 kernel, v2.

Math (per graph g): out_g = sum_v alpha_v H_v,  alpha = softmax_g(e),
  e_v = w_score . tanh(W_proj @ H_v + b_proj).

Key transformations:
 * Global shift: softmax is shift invariant per segment, and
   |e| <= ||w_score||_1 (tanh bounded), so a single global constant
   C = ||w_score||_1 replaces the per-segment max. Then
   out_g = (sum_v a_v [H_v|1])[:D] / (...)[D]  with a_v = exp(e_v - C):
   two segment sums, done as one matmul with a ones-column.
 * Quadrant-windowed segment matmul: segments are grouped into blocks
   of 128 (PSUM accumulator partitions). batch is sorted, so each
   128-node tile spans only a few consecutive segments. The host packs
   nodes so tile t's segments fit [32q(t), 32q(t)+64) for a static
   quadrant schedule q(t); the scatter matmul then uses a [128, 32]
   one-hot lhsT accumulating into PSUM partitions [32q, 32q+32) (PE
   tile_position allows only 32-aligned output bases). Tiles whose
   segments cross into the next quadrant emit one extra matmul (rare).
   The 32-wide one-hot build costs ~1/4 of the full 128-wide one and is
   split between DVE and the otherwise-idle Pool engine.
 * SPMD schedules: one program serves all 8 cores, so q(t) must be
   core-invariant. Blocks are ranked by size; slot k takes ranked
   blocks [8k, 8k+8) (one per core) which share one schedule; per-slot
   tile counts t_k are ragged (saves ~4% DMA vs padding to the max).
 * The projection is computed transposed (h on partitions), so b_proj
   folds into the tanh's per-partition bias for free.
 * Sharding: 8 cores x 16 slots each; cores are fully independent
   (no collectives); host un-permutes the per-core [16,128,128] slabs.
"""

import math

import numpy as np

P = 128                    # partitions / tile node count / D / HS
D = 128
G_SEGS = 16384
SEGS_PER_BLK = 128
NBLK_TOT = G_SEGS // SEGS_PER_BLK   # 128 global blocks
N_CORES = 8
NSLOT = NBLK_TOT // N_CORES         # 16 slots per core
GRP = 28                   # tiles per DMA group
CHUNK = 8                  # tiles per proj-PSUM/tanh chunk
QW = 32                    # quadrant width (PSUM write alignment)

CFG = {
    "ht": "f16",
    "rhs": "f16",
    "act": "f16",
    "oh_dve_mod": 1,       # every Nth one-hot build on DVE, rest on Pool
    "e_op": "pe",          # "pe" | "stt" | "ttr": engine for the score reduce
}

# alpha values are prescaled by exp(PRESCALE_LN) inside the exp bias so the
# smallest per-segment weights stay in f16 normal range; numerator and
# denominator scale together so the final division cancels it exactly.
PRESCALE_LN = 14 * math.log(2.0)

_NP_DT = {"f32": np.float32, "f16": np.float16}


def _np_dt(name):
    if name == "bf16":
        import ml_dtypes
        return ml_dtypes.bfloat16
    return _NP_DT[name]


def _my_dt(name, mybir):
    return {
        "f32": mybir.dt.float32,
        "bf16": mybir.dt.bfloat16,
        "f16": mybir.dt.float16,
    }[name]


def _pack_block(seg_local, q_sched):
    """Greedy pack of one block's nodes onto tiles: tile t may hold nodes
    with local seg in [32*q(t), 32*q(t)+64). Returns list of (pos, take)
    per tile, or None if infeasible."""
    n = len(seg_local)
    pos = 0
    tiles = []
    for t in range(len(q_sched)):
        if pos >= n:
            break
        lo = QW * q_sched[t]
        if seg_local[pos] < lo:
            return None
        hi = np.searchsorted(seg_local, min(lo + 2 * QW, SEGS_PER_BLK),
                             side="left")
        take = int(min(128, hi - pos))
        tiles.append((pos, take))
        pos += take
    if pos < n:
        return None
    return tiles


def _plan(batch):
    """Blocks -> slot plans (quadrant schedules + per-block tilings)."""
    s = np.searchsorted(batch, np.arange(NBLK_TOT + 1, dtype=np.int64)
                        * SEGS_PER_BLK)
    lens = s[1:] - s[:-1]
    order = np.argsort(lens, kind="stable")

    slots = []
    for k in range(NSLOT):
        blks = order[k * N_CORES:(k + 1) * N_CORES]
        maxlen = int(max(1, lens[blks].max()))
        T = int(np.ceil(maxlen / 128)) + 8
        seglists = []
        curves = []
        for b in blks:
            segl = (batch[s[b]:s[b + 1]] - b * SEGS_PER_BLK).astype(np.int64)
            seglists.append(segl)
            sl = segl if len(segl) else np.zeros(1, np.int64)
            idx = np.minimum(np.arange(T) * 128, len(sl) - 1)
            curves.append(sl[idx])
        wmin = np.array(curves).min(axis=0)

        packed = None
        # primary: quadrant schedule from the consensus curve; fallback:
        # all-zero schedule with full-width windows (always feasible)
        for relax in (2, 8, 16, None):
            if relax is None:
                q_sched = np.zeros(T, np.int64)
                full = True
            else:
                w = np.maximum(0, np.minimum(SEGS_PER_BLK - QW, wmin - relax))
                q_sched = np.maximum.accumulate(w // QW).astype(np.int64)
                full = False
            tilings = []
            ok = True
            for segl in seglists:
                tiles = (_pack_block(segl, q_sched) if not full else
                         [(i * 128, int(min(128, len(segl) - i * 128)))
                          for i in range(int(np.ceil(len(segl) / 128)))]
                         or [(0, 0)])
                if tiles is None:
                    ok = False
                    break
                tilings.append(tiles)
            if ok:
                t_k = max(1, max(len(tl) for tl in tilings))
                q_sched = q_sched[:t_k]
                # split flag: any block has a node beyond its tile's quadrant
                split = np.zeros(t_k, bool)
                if not full:
                    for segl, tiles in zip(seglists, tilings):
                        for t, (pos, take) in enumerate(tiles):
                            if take and segl[pos + take - 1] >= \
                                    QW * (q_sched[t] + 1):
                                split[t] = True
                packed = dict(blks=blks, q_sched=q_sched, t_k=t_k,
                              tilings=tilings, split=split, full=full)
                break
        slots.append(packed)
    return s, order, slots


def _prep_inputs(H, batch, W_proj, b_proj, w_score, cfg):
    """Host-side repack. Returns (in_maps, plan_meta)."""
    H = np.ascontiguousarray(H, dtype=np.float32)
    batch = np.asarray(batch).astype(np.int64)
    W_proj = np.asarray(W_proj, dtype=np.float32)
    b_proj = np.asarray(b_proj, dtype=np.float32)
    w_score = np.asarray(w_score, dtype=np.float32)

    c_shift = float(np.abs(w_score).sum())

    s, order, slots = _plan(batch)

    dt_rhs = _np_dt(cfg["rhs"])
    dt_ht = _np_dt(cfg["ht"])
    DC = D + 1                       # H | ones
    H16 = H.astype(dt_rhs)

    in_maps = [dict() for _ in range(N_CORES)]

    for k, sl in enumerate(slots):
        t_k, q_sched = sl["t_k"], sl["q_sched"]
        for c in range(N_CORES):
            b = int(sl["blks"][c])
            tiles = sl["tilings"][c]
            segl = (batch[s[b]:s[b + 1]] - b * SEGS_PER_BLK).astype(np.int64)
            idx = np.full((t_k, 128), -1, np.int64)
            for t, (pos, take) in enumerate(tiles):
                if take:
                    idx[t, :take] = np.arange(s[b] + pos, s[b] + pos + take)
            valid = idx >= 0
            idxc = np.maximum(idx, 0)

            Hg = H16[idxc]
            Hg[~valid] = 0.0
            rhs = np.zeros((t_k, 128, DC), dt_rhs)
            rhs[:, :, :D] = Hg
            rhs[:, :, D] = valid
            bl = np.full((t_k, 128), -1000.0, np.float32)
            for t, (pos, take) in enumerate(tiles):
                if take:
                    bl[t, :take] = segl[pos:pos + take] - QW * q_sched[t]
            m = in_maps[c]
            m[f"rhs{k}"] = np.ascontiguousarray(rhs.transpose(1, 0, 2))
            del rhs
            m[f"bl{k}"] = np.ascontiguousarray(bl.T)
            m[f"ht{k}"] = np.ascontiguousarray(Hg.transpose(2, 0, 1))
            del Hg

    wt = np.ascontiguousarray(W_proj.T.astype(dt_ht))
    wb = np.ascontiguousarray(
        np.broadcast_to(w_score, (P, D)).astype(_np_dt(cfg["act"])))
    # [h, tt, m]: col tt = w_score, other 31 columns zero - each masked
    # e-matmul then zero-fills its whole 32-aligned PSUM region, so the
    # later exp never reads uninitialized PSUM (a HW fault).
    wmask = np.zeros((P, CHUNK, QW), np.float32)
    for tt in range(CHUNK):
        wmask[:, tt, tt] = w_score
    wmask = np.ascontiguousarray(
        wmask.reshape(P, CHUNK * QW).astype(_np_dt(cfg["act"])))
    eye16 = np.zeros((2 * QW, CHUNK), np.float32)
    eye16[0:CHUNK] = np.eye(CHUNK)
    eye16[QW:QW + CHUNK] = np.eye(CHUNK)
    eye16 = np.ascontiguousarray(eye16.astype(_np_dt(cfg["act"])))
    bb = np.ascontiguousarray(b_proj.reshape(P, 1).astype(np.float32))
    iota_lo = np.ascontiguousarray(
        np.broadcast_to(np.arange(QW, dtype=np.float32), (P, QW))
        .astype(dt_rhs))
    iota_hi = np.ascontiguousarray((iota_lo.astype(np.float32) + QW)
                                   .astype(dt_rhs))
    iota_fl = np.ascontiguousarray(
        np.broadcast_to(np.arange(P, dtype=np.float32), (P, P))
        .astype(dt_rhs))
    for c in range(N_CORES):
        in_maps[c]["wt"] = wt
        in_maps[c]["wb"] = wb
        in_maps[c]["wmask"] = wmask
        in_maps[c]["eye16"] = eye16
        in_maps[c]["iota_lo"] = iota_lo
        in_maps[c]["iota_hi"] = iota_hi
        in_maps[c]["iota_fl"] = iota_fl
        in_maps[c]["bb"] = bb

    meta = dict(slots=[dict(t_k=sl["t_k"], q_sched=sl["q_sched"].tolist(),
                            split=sl["split"].tolist(), full=sl["full"])
                       for sl in slots],
                order=order, c_shift=c_shift)
    return in_maps, meta


def _build_program(meta, cfg):
    import concourse.bacc as bacc
    import concourse.mybir as mybir
    import concourse.tile as tile

    f32 = mybir.dt.float32
    dt_ht = _my_dt(cfg["ht"], mybir)
    dt_rhs = _my_dt(cfg["rhs"], mybir)
    dt_act = _my_dt(cfg["act"], mybir)
    c_shift = meta["c_shift"]
    slots = meta["slots"]
    DC = D + 1
    oh_mod = int(cfg["oh_dve_mod"])

    nc = bacc.Bacc(None, target_bir_lowering=False)

    rhs_d, ht_d = [], []
    for k, sl in enumerate(slots):
        rhs_d.append(nc.dram_tensor(f"rhs{k}", [P, sl["t_k"], DC], dt_rhs,
                                    kind="ExternalInput"))
        ht_d.append(nc.dram_tensor(f"ht{k}", [P, sl["t_k"], D], dt_ht,
                                   kind="ExternalInput"))
    bl_d = [nc.dram_tensor(f"bl{k}", [P, sl["t_k"]], f32,
                           kind="ExternalInput")
            for k, sl in enumerate(slots)]
    wt_d = nc.dram_tensor("wt", [P, D], dt_ht, kind="ExternalInput")
    wb_d = nc.dram_tensor("wb", [P, D], dt_act, kind="ExternalInput")
    wm_d = nc.dram_tensor("wmask", [P, CHUNK * QW], dt_act,
                          kind="ExternalInput")
    ey_d = nc.dram_tensor("eye16", [2 * QW, CHUNK], dt_act,
                          kind="ExternalInput")
    ilo_d = nc.dram_tensor("iota_lo", [P, QW], dt_rhs, kind="ExternalInput")
    ihi_d = nc.dram_tensor("iota_hi", [P, QW], dt_rhs, kind="ExternalInput")
    ifl_d = nc.dram_tensor("iota_fl", [P, P], dt_rhs, kind="ExternalInput")
    bb_d = nc.dram_tensor("bb", [P, 1], f32, kind="ExternalInput")
    out_d = nc.dram_tensor("out", [NSLOT, P, D], f32, kind="ExternalOutput")

    with tile.TileContext(nc) as tc:
        with (
            tc.tile_pool(name="const", bufs=1) as constp,
            tc.tile_pool(name="rhsp", bufs=9) as rhsp,
            tc.tile_pool(name="htp", bufs=8) as htp,
            tc.tile_pool(name="midp", bufs=8) as midp,
            tc.tile_pool(name="ohp", bufs=60) as ohp,
            tc.tile_pool(name="blp", bufs=2) as blp,
            tc.tile_pool(name="pp", bufs=2, space="PSUM") as pp,
            tc.tile_pool(name="op", bufs=1, space="PSUM") as op,
            tc.tile_pool(name="ep", bufs=2, space="PSUM") as ep,
            tc.tile_pool(name="tp", bufs=1, space="PSUM") as tp,
        ):
            wt_sb = constp.tile([P, D], dt_ht)
            nc.scalar.dma_start(wt_sb[:], wt_d[:])
            wb_sb = constp.tile([P, D], dt_act)
            nc.scalar.dma_start(wb_sb[:], wb_d[:])
            wm_sb = constp.tile([P, CHUNK, QW], dt_act)
            nc.scalar.dma_start(wm_sb[:], wm_d[:].rearrange(
                "p (t m) -> p t m", t=CHUNK))
            ey_sb = constp.tile([2 * QW, CHUNK], dt_act)
            nc.scalar.dma_start(ey_sb[:], ey_d[:])
            ilo_sb = constp.tile([P, QW], dt_rhs)
            nc.scalar.dma_start(ilo_sb[:], ilo_d[:])
            ihi_sb = constp.tile([P, QW], dt_rhs)
            nc.scalar.dma_start(ihi_sb[:], ihi_d[:])
            ifl_sb = constp.tile([P, P], dt_rhs)
            nc.scalar.dma_start(ifl_sb[:], ifl_d[:])
            zrhs_sb = constp.tile([P, D + 1], dt_rhs)
            nc.gpsimd.memset(zrhs_sb[:], 0.0)
            negc_sb = constp.tile([P, 1], f32)
            nc.gpsimd.memset(negc_sb[:], -float(c_shift) + PRESCALE_LN)
            bb_sb = constp.tile([P, 1], f32)
            nc.scalar.dma_start(bb_sb[:], bb_d[:])

            tmax = max(sl["t_k"] for sl in slots)
            tasks = []
            # largest slots first: the pipeline drain then runs on the
            # smallest slot, shortening the tail
            for k in sorted(range(NSLOT), key=lambda k: -slots[k]["t_k"]):
                for g0 in range(0, slots[k]["t_k"], GRP):
                    tasks.append((k, g0, min(GRP, slots[k]["t_k"] - g0)))

            out_ps_by_slot = {}
            bl_by_slot = {}
            oh_ctr = 0

            def emit_exp_chunk(e_ps_c):
                aT = midp.tile([QW, P], dt_act, tag="aT")
                nc.scalar.activation(
                    aT[:QW], e_ps_c[:QW],
                    mybir.ActivationFunctionType.Exp, bias=negc_sb[:QW])
                return aT

            def emit_weights(pend):
                """transpose+copy of a, then the one-hot builds (1 group
                ahead of the scatter matmuls). Returns oh tile list."""
                nonlocal oh_ctr
                k, g0, gsz, rhs_sb, aTs = pend
                sl = slots[k]
                a_ps = tp.tile([P, GRP], dt_act)
                for ci, c0, csz, aT in aTs:
                    nc.tensor.transpose(a_ps[:, c0:c0 + csz], aT[:csz],
                                        ey_sb[:csz, :csz])
                a16 = midp.tile([P, GRP], f32, tag="a16")
                nc.vector.tensor_copy(a16[:, :gsz], a_ps[:, :gsz])
                bl_sb = bl_by_slot[k]
                ohs = []
                for tt in range(gsz):
                    t = g0 + tt
                    bl_ap = bl_sb[:, t:t + 1]
                    a_ap = a16[:, tt:tt + 1]
                    wide = QW * 4 if sl["full"] else QW
                    iota = ifl_sb if sl["full"] else ilo_sb
                    oh = ohp.tile([P, wide], dt_rhs,
                                  tag="ohf" if sl["full"] else "oh")
                    eng = nc.vector if oh_ctr % oh_mod == 0 else nc.gpsimd
                    oh_ctr += 1
                    eng.tensor_scalar(
                        oh[:], iota[:], bl_ap, a_ap,
                        mybir.AluOpType.is_equal, mybir.AluOpType.mult)
                    oh2 = None
                    q = int(sl["q_sched"][t]) if not sl["full"] else 0
                    if not sl["full"] and bool(sl["split"][t]) and q < 3:
                        oh2 = ohp.tile([P, QW], dt_rhs, tag="oh2")
                        nc.vector.tensor_scalar(
                            oh2[:], ihi_sb[:], bl_ap, a_ap,
                            mybir.AluOpType.is_equal, mybir.AluOpType.mult)
                    ohs.append((oh, oh2))
                return (k, g0, gsz, rhs_sb, ohs)

            def emit_scatter(pend):
                k, g0, gsz, rhs_sb, ohs = pend
                sl = slots[k]
                t_k, q_sched, full = sl["t_k"], sl["q_sched"], sl["full"]
                if g0 == 0:
                    out_ps = op.tile([P, D + 1], f32)
                    out_ps_by_slot[k] = (out_ps, set())
                out_ps, qdone = out_ps_by_slot[k]

                def mm(qbase, qwid, oh, stop):
                    first = qbase not in qdone
                    qdone.add(qbase)
                    nc.tensor.matmul(
                        out_ps[qbase:qbase + qwid, :], oh[:],
                        rhs_sb[:, tt, 0:D + 1],
                        start=first, stop=stop,
                        skip_group_check=True, tile_position=(0, qbase))

                for tt in range(gsz):
                    t = g0 + tt
                    is_last = (t == t_k - 1)
                    oh, oh2 = ohs[tt]
                    if full:
                        mm(0, P, oh, is_last)
                        continue
                    q = int(q_sched[t])
                    mm(QW * q, QW, oh, is_last and oh2 is None)
                    if oh2 is not None:
                        mm(QW * (q + 1), QW, oh2, is_last)
                if g0 + gsz == t_k:
                    if not full:
                        for q in range(4):
                            if QW * q not in qdone:
                                nc.tensor.matmul(
                                    out_ps[QW * q:QW * (q + 1), :],
                                    wt_sb[:, 0:QW], zrhs_sb[:],
                                    start=True, stop=True,
                                    skip_group_check=True,
                                    tile_position=(0, QW * q))
                    elif not qdone:
                        nc.tensor.matmul(out_ps[:], wt_sb[:], zrhs_sb[:],
                                         start=True, stop=True,
                                         skip_group_check=True)
                    den_sb = midp.tile([P, 1], f32, tag="den")
                    nc.vector.tensor_scalar_max(den_sb[:],
                                                out_ps[:, D:D + 1], 1e-12)
                    rec_sb = midp.tile([P, 1], f32, tag="rec")
                    nc.vector.reciprocal(rec_sb[:], den_sb[:])
                    res_sb = midp.tile([P, D], f32, tag="res")
                    nc.vector.tensor_scalar(
                        res_sb[:], out_ps[:, 0:D], rec_sb[:], None,
                        mybir.AluOpType.mult)
                    nc.scalar.dma_start(out_d[k], res_sb[:])

            pend_e = None      # group awaiting weights (exp already done)
            pend_w = None
            pend_w2 = None
            for k, g0, gsz in tasks:
                sl = slots[k]
                if g0 == 0:
                    bl_sb = blp.tile([P, tmax], f32, tag="bl")
                    nc.scalar.dma_start(bl_sb[:, :sl["t_k"]], bl_d[k][:])
                    bl_by_slot[k] = bl_sb
                rhs_sb = rhsp.tile([P, GRP, DC], dt_rhs, tag="rhs")
                nc.sync.dma_start(rhs_sb[:, :gsz],
                                  rhs_d[k][:, g0:g0 + gsz, :])
                ht_sb = htp.tile([P, GRP, D], dt_ht, tag="ht")
                nc.sync.dma_start(ht_sb[:, :gsz],
                                  ht_d[k][:, g0:g0 + gsz, :])
                tanh_cs = []
                for ci, c0 in enumerate(range(0, gsz, CHUNK)):
                    csz = min(CHUNK, gsz - c0)
                    proj_ps = pp.tile([P, CHUNK, D], f32)
                    for tt in range(csz):
                        nc.tensor.matmul(proj_ps[:, tt], wt_sb[:],
                                         ht_sb[:, c0 + tt],
                                         start=True, stop=True)
                    tanh_c = midp.tile([P, CHUNK, D], dt_act, tag="tanh")
                    nc.scalar.activation(
                        tanh_c[:, :csz], proj_ps[:, :csz],
                        mybir.ActivationFunctionType.Tanh, bias=bb_sb[:])
                    tanh_cs.append((c0, csz, tanh_c))
                if pend_w is not None:
                    emit_scatter(pend_w)
                    pend_w = None
                aTs = []
                for ci, (c0, csz, tanh_c) in enumerate(tanh_cs):
                    e_ps_c = ep.tile([QW, P], f32, tag="eps")
                    for tt in range(csz):
                        nc.tensor.matmul(
                            e_ps_c[:, :], wm_sb[:, tt], tanh_c[:, tt],
                            start=(tt == 0), stop=(tt == csz - 1),
                            skip_group_check=True)
                    aTs.append((ci, c0, csz, emit_exp_chunk(e_ps_c)))
                new_e = (k, g0, gsz, rhs_sb, aTs)
                # weights of the previous group are built at the tail of
                # this iteration (their exp is long done); their scatter
                # runs next iteration, filling the PE hole under the tanhs
                pend_w = emit_weights(pend_e) if pend_e is not None else None
                pend_e = new_e
            # drain the pipeline
            for pw in (pend_w2, pend_w):
                if pw is not None:
                    emit_scatter(pw)
            if pend_e is not None:
                emit_scatter(emit_weights(pend_e))
    nc.compile()
    return nc


def kernel(H, batch, W_proj, b_proj, w_score):
    from concourse.bass_utils import run_bass_kernel_spmd

    in_maps, meta = _prep_inputs(H, batch, W_proj, b_proj, w_score, CFG)
    nc = _build_program(meta, CFG)
    res = run_bass_kernel_spmd(nc, in_maps, core_ids=list(range(N_CORES)))
    out = np.empty((G_SEGS, D), np.float32)
    order = meta["order"]
    for c in range(N_CORES):
        slab = res.results[c]["out"]          # [NSLOT, P, D]
        for k in range(NSLOT):
            b = int(order[k * N_CORES + c])
            out[b * P:(b + 1) * P] = slab[k]
    return out.astype(np.float32)
